# revision 27
# baseline (speedup 1.0000x reference)
"""AlphaNet-v1 Trainium2 kernel: windowed stats + global BatchNorm + tiny MLP.

Strategy (data-parallel over batch, 8 cores):
  Phase 1 (device): per-sample raw features (corr/cov/std/z/dec/mean/ret over
    3 windows of 10 days, plus window sum/max/min) computed sample-major on
    DVE, then PE-transposed to feature-major rawT[1024, BS] in DRAM, plus
    per-column sum-of-squares (ScalarE accum on transposed tiles) and
    per-group linear sums for the BatchNorm statistics.
  Host: combine partial sums -> per-column affine (A, C) for all 990 flat
    features (BN is a global affine; gamma=1 so it commutes with max/min),
    fold into W1' = W1*A, b1' = b1 + W1@C.
  Phase 2 (device): h = relu(W1' @ rawT + b1') via chunked PE matmuls with
    features on the contraction axis (no transposes), y = u @ h + c0.
"""

import numpy as np

import concourse.bass as bass
import concourse.bacc as bacc
import concourse.mybir as mybir
from concourse.tile import TileContext
from concourse.bass_utils import run_bass_kernel_spmd

F32 = mybir.dt.float32
ALU = mybir.AluOpType
AX = mybir.AxisListType
AF = mybir.ActivationFunctionType

B = 131072
NCORES = 8
BS = B // NCORES            # 16384 samples per core
S = 8                       # sample-blocks per tile (each 128 samples)
PT = 128
TSAMP = PT * S              # 1024 samples per tile
NT = BS // TSAMP            # 16 tiles
NF, NW, ND = 11, 3, 10
XC = NF * NW * ND           # 330
RC = 1024                   # padded raw feature columns (990 used)
EPS = 1e-5

# base feature groups in F (165 features x 3 windows): feat ranges
GF = [(0, 55), (55, 110), (110, 121), (121, 132), (132, 143), (143, 154), (154, 165)]
GCNT = np.array([55, 55, 11, 11, 11, 11, 11], dtype=np.int64)
# (alpha, beta): true_feature = alpha*raw + beta
GAB = [(1.0, 0.0), (1.0 / 9.0, 0.0), (1.0 / 3.0, 0.0), (0.3, 0.0),
       (1.0 / 55.0, 0.0), (0.1, 0.0), (1.0, -1.0)]
# feature slices in F: corr, cov, std, z, dec, mean, ret
SL_CORR, SL_COV, SL_STD, SL_Z, SL_DEC, SL_MEAN, SL_RET = GF
# pair-block offsets for shifted products d=1..10 within 55-pair packing
OFFD = np.cumsum([0] + [11 - d for d in range(1, 11)]).tolist()  # OFFD[d-1] = start


def _feat_perm():
    """map my 165-feature index -> reference feature index (triu pair order)."""
    iu, ju = np.triu_indices(NF, k=1)
    ref_of_pair = {(i, j): k for k, (i, j) in enumerate(zip(iu, ju))}
    fmap = np.arange(165)
    for d in range(1, 11):
        for f in range(NF - d):
            mine = OFFD[d - 1] + f
            ref = ref_of_pair[(f, f + d)]
            fmap[mine] = ref           # corr block
            fmap[55 + mine] = 55 + ref  # cov block
    return fmap


FMAP = _feat_perm()
# my raw col -> reference flat col (990)
COLMAP = np.zeros(990, dtype=np.int64)
for _fm in range(165):
    for _w in range(3):
        COLMAP[3 * _fm + _w] = 3 * FMAP[_fm] + _w
for _si in range(3):
    for _fm in range(165):
        COLMAP[495 + 165 * _si + _fm] = 495 + 165 * _si + FMAP[_fm]

# my raw col -> BN group id for the squared-sum reduction (base groups 0..6,
# window-stat groups 7 + 7*si + g), -1 for pad
gof = np.concatenate([np.full(GCNT[g], g) for g in range(7)])  # feat->group
COLGRP = np.full(RC, -1, dtype=np.int64)
COLGRP[0:495] = np.repeat(gof, 3)
for _si in range(3):
    COLGRP[495 + 165 * _si:660 + 165 * _si] = 7 + 7 * _si + gof


def build_phase1():
    nc = bacc.Bacc()
    x_in = nc.dram_tensor("x_in", [BS, XC], F32, kind="ExternalInput")
    wfull_in = nc.dram_tensor("wfull_in", [PT, S * XC], F32, kind="ExternalInput")
    id_in = nc.dram_tensor("id_in", [PT, PT], F32, kind="ExternalInput")
    rawt_out = nc.dram_tensor("rawt_out", [RC, BS], F32, kind="ExternalOutput")
    psq_out = nc.dram_tensor("psq_out", [PT, NT, 8], F32, kind="ExternalOutput")
    ps_out = nc.dram_tensor("ps_out", [PT, NT, 32], F32, kind="ExternalOutput")

    # sample id = t*1024 + s*128 + p  (contiguous 128-sample runs per (t,s))
    x_t = x_in.rearrange("(t s p) c -> t p s c", t=NT, s=S, p=PT)
    # iteration order (cp, cc, j) matches the SBUF-side [p, cc, j] layout
    rt = rawt_out.rearrange("(cc cp) (t s j) -> t s cp cc j", cc=8, cp=PT, t=NT, s=S)

    with TileContext(nc) as tc:
        with tc.tile_pool(name="cst", bufs=1) as cp, \
             tc.tile_pool(name="io", bufs=2) as iop, \
             tc.tile_pool(name="wk", bufs=1) as wp, \
             tc.tile_pool(name="big", bufs=2) as bp, \
             tc.tile_pool(name="ft", bufs=1) as fp_, \
             tc.tile_pool(name="tp", bufs=2, space="PSUM") as pp:

            wful = cp.tile([PT, S * XC], F32)
            nc.sync.dma_start(wful[:], wfull_in[:, :])
            IDT = cp.tile([PT, PT], F32)
            nc.sync.dma_start(IDT[:], id_in[:, :])
            SCR = cp.tile([PT, S * 55 * NW], F32)
            PSQ = cp.tile([PT, NT, 8], F32)
            PSALL = cp.tile([PT, NT, 32], F32)
            Xt = []
            X0 = iop.tile([PT, S * XC], F32, tag="X")
            nc.sync.dma_start(X0.rearrange("p (s c) -> p s c", s=S), x_t[0])
            Xt.append(X0)
            for t in range(NT):
                if t + 1 < NT:
                    Xn = iop.tile([PT, S * XC], F32, tag="X")
                    nc.sync.dma_start(Xn.rearrange("p (s c) -> p s c", s=S),
                                      x_t[t + 1])
                    Xt.append(Xn)
                X = Xt[t]
                Xv = X.rearrange("p (s f w d) -> p s f w d", s=S, f=NF, w=NW, d=ND)
                X4 = X.rearrange("p (s f wd) -> p s f wd", s=S, f=NF, wd=NW * ND)

                RAW = fp_.tile([PT, S * RC], F32, tag="RAW")
                RV = RAW.rearrange("p (s c) -> p s c", s=S)
                F3 = RV[:, :, 0:495].rearrange("p s (f w) -> p s f w", w=NW)
                nc.vector.memset(RV[:, :, 990:RC], 0.0)

                mean3 = F3[:, :, SL_MEAN[0]:SL_MEAN[1], :]
                nc.vector.tensor_reduce(mean3, Xv, axis=AX.X, op=ALU.add)

                # squares -> s2 (squares on ScalarE, reduce on DVE)
                SQ = bp.tile([PT, S * XC], F32, tag="BIG")
                nc.scalar.activation(SQ[:], X[:], AF.Square)
                s2t = wp.tile([PT, S, NF, NW], F32, tag="s2t")
                nc.vector.tensor_reduce(
                    s2t[:], SQ.rearrange("p (s f w d) -> p s f w d",
                                         s=S, f=NF, w=NW, d=ND),
                    axis=AX.X, op=ALU.add)

                # shifted pair products d=1..10 -> SS [p,S,55,3]
                SS = wp.tile([PT, S, 55, NW], F32, tag="SS")
                for d in range(1, 11):
                    o = OFFD[d - 1]
                    PD = bp.tile([PT, S, NF - d, NW * ND], F32, tag="BIG")
                    nc.vector.tensor_tensor(PD[:], X4[:, :, 0:NF - d, :],
                                            X4[:, :, d:NF, :], ALU.mult)
                    nc.vector.tensor_reduce(
                        SS[:, :, o:o + NF - d, :],
                        PD.rearrange("p s f (w d) -> p s f w d", w=NW),
                        axis=AX.X, op=ALU.add)

                # scaled mean (0.1*mean') -> pre-scaled pair products
                MSC = wp.tile([PT, S, NF, NW], F32, tag="MSC")
                nc.vector.tensor_scalar_mul(MSC[:], mean3, 0.1)
                MM = wp.tile([PT, S, 55, NW], F32, tag="MM")
                for d in range(1, 11):
                    o = OFFD[d - 1]
                    nc.vector.tensor_tensor(MM[:, :, o:o + NF - d, :],
                                            MSC[:, :, 0:NF - d, :],
                                            F3[:, :, SL_MEAN[0] + d:SL_MEAN[1], :],
                                            ALU.mult)
                # var' = s2 - 0.1*mean'^2 ; cov' = SS - 0.1*MM
                VT = wp.tile([PT, S, NF, NW], F32, tag="VT")
                nc.vector.tensor_tensor(VT[:], MSC[:], mean3, ALU.mult)
                VARP = wp.tile([PT, S, NF, NW], F32, tag="VARP")
                nc.vector.tensor_tensor(VARP[:], s2t[:], VT[:], ALU.subtract)
                covF = F3[:, :, SL_COV[0]:SL_COV[1], :]
                nc.vector.tensor_tensor(covF, SS[:], MM[:], ALU.subtract)

                # dec = sum(x * d)
                DW = bp.tile([PT, S * XC], F32, tag="BIG")
                nc.vector.tensor_tensor(DW[:], X[:], wful[:], ALU.mult)
                nc.vector.tensor_reduce(
                    F3[:, :, SL_DEC[0]:SL_DEC[1], :],
                    DW.rearrange("p (s f w d) -> p s f w d", s=S, f=NF, w=NW, d=ND),
                    axis=AX.X, op=ALU.add)

                # std_raw = sqrt(var'), rstd = 1/std_raw
                stdF = F3[:, :, SL_STD[0]:SL_STD[1], :]
                nc.scalar.sqrt(stdF, VARP[:])
                RSTD = wp.tile([PT, S, NF, NW], F32, tag="RSTD")
                nc.vector.reciprocal(RSTD[:], stdF)

                # corr = cov' * rstd_i * rstd_j
                RR = wp.tile([PT, S, 55, NW], F32, tag="RR")
                for d in range(1, 11):
                    o = OFFD[d - 1]
                    nc.vector.tensor_tensor(RR[:, :, o:o + NF - d, :],
                                            RSTD[:, :, 0:NF - d, :],
                                            RSTD[:, :, d:NF, :], ALU.mult)
                nc.vector.tensor_tensor(F3[:, :, SL_CORR[0]:SL_CORR[1], :],
                                        covF, RR[:], ALU.mult)

                # z = mean' * rstd
                nc.vector.tensor_tensor(F3[:, :, SL_Z[0]:SL_Z[1], :],
                                        mean3, RSTD[:], ALU.mult)

                # ret = x9 / x0
                R0 = wp.tile([PT, S, NF, NW], F32, tag="R0")
                nc.vector.reciprocal(R0[:], Xv[:, :, :, :, 0])
                nc.vector.tensor_tensor(F3[:, :, SL_RET[0]:SL_RET[1], :],
                                        R0[:], Xv[:, :, :, :, 9], ALU.mult)

                # window sum/max/min over the 3 windows -> raw cols 495:990
                T01 = wp.tile([PT, S, 165], F32, tag="T01")
                for si, op in enumerate([ALU.add, ALU.max, ALU.min]):
                    nc.vector.tensor_tensor(T01[:], F3[:, :, :, 0], F3[:, :, :, 1], op)
                    nc.vector.tensor_tensor(RV[:, :, 495 + 165 * si:495 + 165 * (si + 1)],
                                            T01[:], F3[:, :, :, 2], op)

                # linear partial sums per group (DVE, per-partition partials)
                PS = PSALL[:, t, :]
                for si in range(3):
                    base = 495 + 165 * si
                    for g, (a, b) in enumerate(GF):
                        seg = RV[:, :, base + a:base + b]
                        nc.vector.tensor_reduce(PS[:, 7 * si + g:7 * si + g + 1],
                                                seg, axis=AX.XY, op=ALU.add)

                # transpose RAW -> feature-major, evac, per-column sums, DMA
                FT = fp_.tile([PT, S, 8, PT], F32, tag="FT")
                for s in range(S):
                    TP = pp.tile([PT, 8 * PT], F32, tag="TP")
                    for cc in range(8):
                        nc.tensor.transpose(TP[:, cc * PT:(cc + 1) * PT],
                                            RV[:, s, cc * PT:(cc + 1) * PT], IDT[:])
                    nc.scalar.copy(FT[:, s, :, :].rearrange("p a b -> p (a b)"), TP[:])
                    nc.sync.dma_start(rt[t, s], FT[:, s, :, :])
                for cc in range(8):
                    scr = SCR[:, 0:S * PT].rearrange("p (s j) -> p s j", s=S)
                    nc.scalar.activation(scr, FT[:, :, cc, :], AF.Square,
                                         accum_out=PSQ[:, t, cc:cc + 1])
            nc.vector.memset(PSALL[:, :, 21:32], 0.0)
            nc.sync.dma_start(psq_out[:, :, :], PSQ[:])
            nc.sync.dma_start(ps_out[:, :, :], PSALL[:])
    return nc


def build_phase2():
    nc = bacc.Bacc()
    rawt_in = nc.dram_tensor("rawt_in", [RC, BS], F32, kind="ExternalInput")
    w1t_in = nc.dram_tensor("w1t_in", [RC, 32], F32, kind="ExternalInput")
    b1_in = nc.dram_tensor("b1_in", [32, 1], F32, kind="ExternalInput")
    u_in = nc.dram_tensor("u_in", [32, 1], F32, kind="ExternalInput")
    c0_in = nc.dram_tensor("c0_in", [1, 1], F32, kind="ExternalInput")
    y_out = nc.dram_tensor("y_out", [1, BS], F32, kind="ExternalOutput")

    NB = BS // 512  # 32 blocks of 512 samples
    rtb = rawt_in.rearrange("(cc cp) (n j) -> n cp cc j", cc=8, cp=PT, n=NB)

    with TileContext(nc) as tc:
        with tc.tile_pool(name="cst", bufs=1) as cp, \
             tc.tile_pool(name="sb", bufs=3) as sp, \
             tc.tile_pool(name="ps", bufs=4, space="PSUM") as pp:
            W1S = cp.tile([PT, 8 * 32], F32)
            W1Sv = W1S.rearrange("p (c m) -> p c m", c=8)
            nc.sync.dma_start(W1Sv, w1t_in.rearrange("(c p) m -> p c m", c=8, p=PT))
            B1T = cp.tile([32, 1], F32)
            nc.sync.dma_start(B1T[:], b1_in[:, :])
            UT = cp.tile([32, 1], F32)
            nc.sync.dma_start(UT[:], u_in[:, :])
            C0T = cp.tile([1, 1], F32)
            nc.sync.dma_start(C0T[:], c0_in[:, :])

            for n in range(NB):
                RT = sp.tile([PT, 8, 512], F32, tag="RT")
                nc.gpsimd.dma_start(RT[:], rtb[n])
                HP = pp.tile([32, 512], F32, tag="HP")
                for cc in range(8):
                    nc.tensor.matmul(HP[:], W1Sv[:, cc, :], RT[:, cc, :],
                                     start=(cc == 0), stop=(cc == 7))
                HS = sp.tile([32, 512], F32, tag="HS")
                nc.scalar.activation(HS[:], HP[:], AF.Relu, bias=B1T[:, 0:1], scale=1.0)
                OP = pp.tile([1, 512], F32, tag="OP")
                nc.tensor.matmul(OP[:], UT[:], HS[:], start=True, stop=True)
                OS = sp.tile([1, 512], F32, tag="OS")
                nc.vector.tensor_scalar(OS[:], OP[:], C0T[0:1, 0:1], None, ALU.add)
                nc.gpsimd.dma_start(y_out[0:1, n * 512:(n + 1) * 512], OS[:])
    return nc


_CACHE = {}
LAST_EXEC_NS = {}


def _run(nc, in_maps, **kw):
    import os
    tr = os.environ.get("KTRACE", "") == "1"
    if tr:
        kw.setdefault("trace", True)
    return run_bass_kernel_spmd(nc, in_maps, **kw)


def _get_kernels():
    if "p1" not in _CACHE:
        _CACHE["p1"] = build_phase1()
        _CACHE["p1"].finalize()
        _CACHE["p2"] = build_phase2()
        _CACHE["p2"].finalize()
    return _CACHE["p1"], _CACHE["p2"]


def kernel(x, gamma, beta, W1, b1, W2, b2, w_scale, b_scale):
    x = np.asarray(x, dtype=np.float32)
    W1 = np.asarray(W1, np.float32); b1 = np.asarray(b1, np.float32)
    W2 = np.asarray(W2, np.float32); b2 = np.asarray(b2, np.float32)
    gamma_f = float(np.asarray(gamma).reshape(-1)[0])
    beta_f = float(np.asarray(beta).reshape(-1)[0])
    wsc = float(np.asarray(w_scale).reshape(-1)[0])
    bsc = float(np.asarray(b_scale).reshape(-1)[0])

    nc1, nc2 = _get_kernels()
    xs = np.ascontiguousarray(x.reshape(B, XC))
    wbase = np.tile(np.arange(1, 11, dtype=np.float32), NF * NW)  # [330]
    wfull = np.tile(wbase, (PT, S))
    ident = np.eye(PT, dtype=np.float32)

    in1 = [{"x_in": xs[c * BS:(c + 1) * BS], "wfull_in": wfull, "id_in": ident}
           for c in range(NCORES)]
    r1 = _run(nc1, in1, core_ids=list(range(NCORES)))
    LAST_EXEC_NS["p1"] = r1.exec_time_ns
    rawts = [r["rawt_out"] for r in r1.results]
    # per-column squared sums (col id = cc*128 + p) + per-group linear sums
    csq = np.zeros(RC, np.float64)
    PL = np.zeros(32, np.float64)
    for r in r1.results:
        csq += r["psq_out"].astype(np.float64).sum(axis=1).T.reshape(-1)
        PL += r["ps_out"].astype(np.float64).sum(axis=(0, 1))
    P = np.zeros(64, np.float64)
    P[7:28] = PL[0:21]
    for g in range(28):
        P[g if g < 7 else 21 + g] = csq[COLGRP == g].sum()

    # base group BN affines
    A_base = np.zeros(7); C_base = np.zeros(7)
    for g in range(7):
        alpha, bet = GAB[g]
        N = float(B * GCNT[g] * 3)
        S1 = P[7 + g]          # sum of raw (= sum of wsum over group)
        S2 = P[g]              # sum of raw^2
        mT = (alpha * S1 + bet * N) / N
        e2 = (alpha * alpha * S2 + 2 * alpha * bet * S1 + bet * bet * N) / N
        v = e2 - mT * mT
        a = gamma_f / np.sqrt(v + EPS)
        c = beta_f - a * mT
        A_base[g] = a * alpha
        C_base[g] = a * bet + c

    # second-level BN affines (wsum/3, wmax, wmin; p1 = groups 0..5, p2 = {6})
    A2 = np.zeros((3, 7)); C2 = np.zeros((3, 7))
    for si in range(3):
        k = A_base * (1.0 / 3.0 if si == 0 else 1.0)
        off = C_base
        S1g = P[7 + 7 * si:14 + 7 * si].copy()
        S2g = P[28 + 7 * si:35 + 7 * si].copy()
        for grp_set, idxs in (("p1", range(6)), ("p2", [6])):
            Ntot = float(B * sum(GCNT[i] for i in idxs))
            m = sum(k[i] * S1g[i] + B * GCNT[i] * off[i] for i in idxs) / Ntot
            e2 = sum(k[i] ** 2 * S2g[i] + 2 * k[i] * off[i] * S1g[i]
                     + B * GCNT[i] * off[i] ** 2 for i in idxs) / Ntot
            v = e2 - m * m
            a2 = gamma_f / np.sqrt(v + EPS)
            c2 = beta_f - a2 * m
            for i in idxs:
                A2[si, i] = a2 * k[i]
                C2[si, i] = a2 * off[i] + c2

    # per-column affine over the 990 raw columns
    A = np.zeros(990); C = np.zeros(990)
    A[0:495] = np.repeat(A_base[gof], 3); C[0:495] = np.repeat(C_base[gof], 3)
    for si in range(3):
        A[495 + 165 * si:660 + 165 * si] = A2[si, gof]
        C[495 + 165 * si:660 + 165 * si] = C2[si, gof]

    W1e = W1[:, COLMAP]
    W1A = np.zeros((32, RC), np.float32)
    W1A[:30, :990] = W1e * A[None, :].astype(np.float32)
    b1p = np.zeros((32, 1), np.float32)
    b1p[:30, 0] = b1 + W1e @ C.astype(np.float32)
    u = np.zeros((32, 1), np.float32)
    u[:30, 0] = wsc * W2[0]
    c0 = np.float32(wsc * float(b2[0]) + bsc)

    in2 = [{"rawt_in": rawts[c], "w1t_in": np.ascontiguousarray(W1A.T),
            "b1_in": b1p, "u_in": u, "c0_in": np.array([[c0]], np.float32)}
           for c in range(NCORES)]
    r2 = _run(nc2, in2, core_ids=list(range(NCORES)))
    LAST_EXEC_NS["p2"] = r2.exec_time_ns
    # sample id within core = t*1024 + s*128 + p == linear index (identity)
    y = np.concatenate([r["y_out"][0] for r in r2.results])
    return y.astype(np.float32)


# revision 32
# speedup vs baseline: 1.2630x; 1.2630x over previous
"""AlphaNet-v1 Trainium2 kernel: windowed stats + global BatchNorm + tiny MLP.

Strategy (data-parallel over batch, 8 cores):
  Phase 1 (device): per-sample raw features (corr/cov/std/z/dec/mean/ret over
    3 windows of 10 days, plus window sum/max/min) computed sample-major on
    DVE, then PE-transposed to feature-major rawT[1024, BS] in DRAM, plus
    per-column sum-of-squares (ScalarE accum on transposed tiles) and
    per-group linear sums for the BatchNorm statistics.
  Host: combine partial sums -> per-column affine (A, C) for all 990 flat
    features (BN is a global affine; gamma=1 so it commutes with max/min),
    fold into W1' = W1*A, b1' = b1 + W1@C.
  Phase 2 (device): h = relu(W1' @ rawT + b1') via chunked PE matmuls with
    features on the contraction axis (no transposes), y = u @ h + c0.
"""

import numpy as np

import concourse.bass as bass
import concourse.bacc as bacc
import concourse.mybir as mybir
from concourse.tile import TileContext
from concourse.bass_utils import run_bass_kernel_spmd

F32 = mybir.dt.float32
ALU = mybir.AluOpType
AX = mybir.AxisListType
AF = mybir.ActivationFunctionType

B = 131072
NCORES = 8
BS = B // NCORES            # 16384 samples per core
S = 8                       # sample-blocks per tile (each 128 samples)
PT = 128
TSAMP = PT * S              # 1024 samples per tile
NT = BS // TSAMP            # 16 tiles
NF, NW, ND = 11, 3, 10
XC = NF * NW * ND           # 330
RC = 1024                   # padded raw feature columns (990 used)
EPS = 1e-5

# base feature groups in F (165 features x 3 windows): feat ranges
GF = [(0, 55), (55, 110), (110, 121), (121, 132), (132, 143), (143, 154), (154, 165)]
GCNT = np.array([55, 55, 11, 11, 11, 11, 11], dtype=np.int64)
# (alpha, beta): true_feature = alpha*raw + beta
GAB = [(1.0, 0.0), (1.0 / 9.0, 0.0), (1.0 / 3.0, 0.0), (0.3, 0.0),
       (1.0 / 55.0, 0.0), (0.1, 0.0), (1.0, -1.0)]
# feature slices in F: corr, cov, std, z, dec, mean, ret
SL_CORR, SL_COV, SL_STD, SL_Z, SL_DEC, SL_MEAN, SL_RET = GF
# pair-block offsets for shifted products d=1..10 within 55-pair packing
OFFD = np.cumsum([0] + [11 - d for d in range(1, 11)]).tolist()  # OFFD[d-1] = start


def _feat_perm():
    """map my 165-feature index -> reference feature index (triu pair order)."""
    iu, ju = np.triu_indices(NF, k=1)
    ref_of_pair = {(i, j): k for k, (i, j) in enumerate(zip(iu, ju))}
    fmap = np.arange(165)
    for d in range(1, 11):
        for f in range(NF - d):
            mine = OFFD[d - 1] + f
            ref = ref_of_pair[(f, f + d)]
            fmap[mine] = ref           # corr block
            fmap[55 + mine] = 55 + ref  # cov block
    return fmap


FMAP = _feat_perm()
# my raw col -> reference flat col (990)
COLMAP = np.zeros(990, dtype=np.int64)
for _fm in range(165):
    for _w in range(3):
        COLMAP[3 * _fm + _w] = 3 * FMAP[_fm] + _w
for _si in range(3):
    for _fm in range(165):
        COLMAP[495 + 165 * _si + _fm] = 495 + 165 * _si + FMAP[_fm]

# my raw col -> BN group id for the squared-sum reduction (base groups 0..6,
# window-stat groups 7 + 7*si + g), -1 for pad
gof = np.concatenate([np.full(GCNT[g], g) for g in range(7)])  # feat->group
COLGRP = np.full(RC, -1, dtype=np.int64)
COLGRP[0:495] = np.repeat(gof, 3)
for _si in range(3):
    COLGRP[495 + 165 * _si:660 + 165 * _si] = 7 + 7 * _si + gof


def build_phase1():
    nc = bacc.Bacc()
    x_in = nc.dram_tensor("x_in", [BS, XC], F32, kind="ExternalInput")
    wfull_in = nc.dram_tensor("wfull_in", [PT, S * XC], F32, kind="ExternalInput")
    id_in = nc.dram_tensor("id_in", [PT, PT], F32, kind="ExternalInput")
    rawt_out = nc.dram_tensor("rawt_out", [RC, BS], F32, kind="ExternalOutput")
    psq_out = nc.dram_tensor("psq_out", [PT, NT, 8], F32, kind="ExternalOutput")
    psl_out = nc.dram_tensor("psl_out", [PT, NT, 8], F32, kind="ExternalOutput")

    # sample id = t*1024 + s*128 + p  (contiguous 128-sample runs per (t,s))
    x_t = x_in.rearrange("(t s p) c -> t p s c", t=NT, s=S, p=PT)
    # iteration order (cp, cc, j) matches the SBUF-side [p, cc, j] layout
    rt = rawt_out.rearrange("(cc cp) (t s j) -> t s cp cc j", cc=8, cp=PT, t=NT, s=S)

    with TileContext(nc) as tc:
        with tc.tile_pool(name="cst", bufs=1) as cp, \
             tc.tile_pool(name="io", bufs=2) as iop, \
             tc.tile_pool(name="wk", bufs=1) as wp, \
             tc.tile_pool(name="big", bufs=2) as bp, \
             tc.tile_pool(name="ft", bufs=1) as fp_, \
             tc.tile_pool(name="tp", bufs=2, space="PSUM") as pp:

            wful = cp.tile([PT, S * XC], F32)
            nc.sync.dma_start(wful[:], wfull_in[:, :])
            IDT = cp.tile([PT, PT], F32)
            nc.sync.dma_start(IDT[:], id_in[:, :])
            SCR = cp.tile([PT, S * 55 * NW], F32)
            PSQ = cp.tile([PT, NT, 8], F32)
            PSL = cp.tile([PT, NT, 8], F32)
            Xt = []
            X0 = iop.tile([PT, S * XC], F32, tag="X")
            nc.sync.dma_start(X0.rearrange("p (s c) -> p s c", s=S), x_t[0])
            Xt.append(X0)
            for t in range(NT):
                if t + 1 < NT:
                    Xn = iop.tile([PT, S * XC], F32, tag="X")
                    nc.sync.dma_start(Xn.rearrange("p (s c) -> p s c", s=S),
                                      x_t[t + 1])
                    Xt.append(Xn)
                X = Xt[t]
                Xv = X.rearrange("p (s f w d) -> p s f w d", s=S, f=NF, w=NW, d=ND)
                X4 = X.rearrange("p (s f wd) -> p s f wd", s=S, f=NF, wd=NW * ND)

                RAW = fp_.tile([PT, S * RC], F32, tag="RAW")
                RV = RAW.rearrange("p (s c) -> p s c", s=S)
                F3 = RV[:, :, 0:495].rearrange("p s (f w) -> p s f w", w=NW)
                nc.vector.memset(RV[:, :, 990:RC], 0.0)

                mean3 = F3[:, :, SL_MEAN[0]:SL_MEAN[1], :]
                nc.vector.tensor_reduce(mean3, Xv, axis=AX.X, op=ALU.add)

                # squares -> s2 (squares on ScalarE, reduce on DVE)
                SQ = bp.tile([PT, S * XC], F32, tag="BIG")
                nc.scalar.activation(SQ[:], X[:], AF.Square)
                s2t = wp.tile([PT, S, NF, NW], F32, tag="s2t")
                nc.vector.tensor_reduce(
                    s2t[:], SQ.rearrange("p (s f w d) -> p s f w d",
                                         s=S, f=NF, w=NW, d=ND),
                    axis=AX.X, op=ALU.add)

                # shifted pair products d=1..10 -> SS [p,S,55,3]
                SS = wp.tile([PT, S, 55, NW], F32, tag="SS")
                for d in range(1, 11):
                    o = OFFD[d - 1]
                    PD = bp.tile([PT, S, NF - d, NW * ND], F32, tag="BIG")
                    nc.vector.tensor_tensor(PD[:], X4[:, :, 0:NF - d, :],
                                            X4[:, :, d:NF, :], ALU.mult)
                    nc.vector.tensor_reduce(
                        SS[:, :, o:o + NF - d, :],
                        PD.rearrange("p s f (w d) -> p s f w d", w=NW),
                        axis=AX.X, op=ALU.add)

                # scaled mean (0.1*mean') -> pre-scaled pair products
                MSC = wp.tile([PT, S, NF, NW], F32, tag="MSC")
                nc.vector.tensor_scalar_mul(MSC[:], mean3, 0.1)
                MM = wp.tile([PT, S, 55, NW], F32, tag="MM")
                for d in range(1, 11):
                    o = OFFD[d - 1]
                    nc.vector.tensor_tensor(MM[:, :, o:o + NF - d, :],
                                            MSC[:, :, 0:NF - d, :],
                                            F3[:, :, SL_MEAN[0] + d:SL_MEAN[1], :],
                                            ALU.mult)
                # var' = s2 - 0.1*mean'^2 ; cov' = SS - 0.1*MM
                VT = wp.tile([PT, S, NF, NW], F32, tag="VT")
                nc.vector.tensor_tensor(VT[:], MSC[:], mean3, ALU.mult)
                VARP = wp.tile([PT, S, NF, NW], F32, tag="VARP")
                nc.vector.tensor_tensor(VARP[:], s2t[:], VT[:], ALU.subtract)
                covF = F3[:, :, SL_COV[0]:SL_COV[1], :]
                nc.vector.tensor_tensor(covF, SS[:], MM[:], ALU.subtract)

                # dec = sum(x * d)
                DW = bp.tile([PT, S * XC], F32, tag="BIG")
                nc.vector.tensor_tensor(DW[:], X[:], wful[:], ALU.mult)
                nc.vector.tensor_reduce(
                    F3[:, :, SL_DEC[0]:SL_DEC[1], :],
                    DW.rearrange("p (s f w d) -> p s f w d", s=S, f=NF, w=NW, d=ND),
                    axis=AX.X, op=ALU.add)

                # std_raw = sqrt(var'), rstd = 1/std_raw
                stdF = F3[:, :, SL_STD[0]:SL_STD[1], :]
                nc.scalar.sqrt(stdF, VARP[:])
                RSTD = wp.tile([PT, S, NF, NW], F32, tag="RSTD")
                nc.vector.reciprocal(RSTD[:], stdF)

                # corr = cov' * rstd_i * rstd_j
                RR = wp.tile([PT, S, 55, NW], F32, tag="RR")
                for d in range(1, 11):
                    o = OFFD[d - 1]
                    nc.vector.tensor_tensor(RR[:, :, o:o + NF - d, :],
                                            RSTD[:, :, 0:NF - d, :],
                                            RSTD[:, :, d:NF, :], ALU.mult)
                nc.vector.tensor_tensor(F3[:, :, SL_CORR[0]:SL_CORR[1], :],
                                        covF, RR[:], ALU.mult)

                # z = mean' * rstd
                nc.vector.tensor_tensor(F3[:, :, SL_Z[0]:SL_Z[1], :],
                                        mean3, RSTD[:], ALU.mult)

                # ret = x9 / x0
                R0 = wp.tile([PT, S, NF, NW], F32, tag="R0")
                nc.vector.reciprocal(R0[:], Xv[:, :, :, :, 0])
                nc.vector.tensor_tensor(F3[:, :, SL_RET[0]:SL_RET[1], :],
                                        R0[:], Xv[:, :, :, :, 9], ALU.mult)

                # window sum/max/min over the 3 windows -> raw cols 495:990
                T01 = wp.tile([PT, S, 165], F32, tag="T01")
                for si, op in enumerate([ALU.add, ALU.max, ALU.min]):
                    nc.vector.tensor_tensor(T01[:], F3[:, :, :, 0], F3[:, :, :, 1], op)
                    nc.vector.tensor_tensor(RV[:, :, 495 + 165 * si:495 + 165 * (si + 1)],
                                            T01[:], F3[:, :, :, 2], op)

                # transpose RAW -> feature-major, evac, per-column sums, DMA
                FT = fp_.tile([PT, S, 8, PT], F32, tag="FT")
                for s in range(S):
                    TP = pp.tile([PT, 8 * PT], F32, tag="TP")
                    for cc in range(8):
                        nc.tensor.transpose(TP[:, cc * PT:(cc + 1) * PT],
                                            RV[:, s, cc * PT:(cc + 1) * PT], IDT[:])
                    nc.scalar.copy(FT[:, s, :, :].rearrange("p a b -> p (a b)"), TP[:])
                    nc.sync.dma_start(rt[t, s], FT[:, s, :, :])
                for cc in range(8):
                    scr = SCR[:, 0:S * PT].rearrange("p (s j) -> p s j", s=S)
                    nc.scalar.activation(scr, FT[:, :, cc, :], AF.Square,
                                         accum_out=PSQ[:, t, cc:cc + 1])
                    nc.scalar.activation(scr, FT[:, :, cc, :], AF.Copy,
                                         accum_out=PSL[:, t, cc:cc + 1])
            nc.sync.dma_start(psq_out[:, :, :], PSQ[:])
            nc.sync.dma_start(psl_out[:, :, :], PSL[:])
    return nc


def build_phase2():
    nc = bacc.Bacc()
    rawt_in = nc.dram_tensor("rawt_in", [RC, BS], F32, kind="ExternalInput")
    w1t_in = nc.dram_tensor("w1t_in", [RC, 32], F32, kind="ExternalInput")
    b1_in = nc.dram_tensor("b1_in", [32, 1], F32, kind="ExternalInput")
    u_in = nc.dram_tensor("u_in", [32, 1], F32, kind="ExternalInput")
    c0_in = nc.dram_tensor("c0_in", [1, 1], F32, kind="ExternalInput")
    y_out = nc.dram_tensor("y_out", [1, BS], F32, kind="ExternalOutput")

    NB = BS // 512  # 32 blocks of 512 samples
    rtb = rawt_in.rearrange("(cc cp) (n j) -> n cp cc j", cc=8, cp=PT, n=NB)

    with TileContext(nc) as tc:
        with tc.tile_pool(name="cst", bufs=1) as cp, \
             tc.tile_pool(name="sb", bufs=3) as sp, \
             tc.tile_pool(name="ps", bufs=4, space="PSUM") as pp:
            W1S = cp.tile([PT, 8 * 32], F32)
            W1Sv = W1S.rearrange("p (c m) -> p c m", c=8)
            nc.sync.dma_start(W1Sv, w1t_in.rearrange("(c p) m -> p c m", c=8, p=PT))
            B1T = cp.tile([32, 1], F32)
            nc.sync.dma_start(B1T[:], b1_in[:, :])
            UT = cp.tile([32, 1], F32)
            nc.sync.dma_start(UT[:], u_in[:, :])
            C0T = cp.tile([1, 1], F32)
            nc.sync.dma_start(C0T[:], c0_in[:, :])

            for n in range(NB):
                RT = sp.tile([PT, 8, 512], F32, tag="RT")
                nc.gpsimd.dma_start(RT[:], rtb[n])
                HP = pp.tile([32, 512], F32, tag="HP")
                for cc in range(8):
                    nc.tensor.matmul(HP[:], W1Sv[:, cc, :], RT[:, cc, :],
                                     start=(cc == 0), stop=(cc == 7))
                HS = sp.tile([32, 512], F32, tag="HS")
                nc.scalar.activation(HS[:], HP[:], AF.Relu, bias=B1T[:, 0:1], scale=1.0)
                OP = pp.tile([1, 512], F32, tag="OP")
                nc.tensor.matmul(OP[:], UT[:], HS[:], start=True, stop=True)
                OS = sp.tile([1, 512], F32, tag="OS")
                nc.vector.tensor_scalar(OS[:], OP[:], C0T[0:1, 0:1], None, ALU.add)
                nc.gpsimd.dma_start(y_out[0:1, n * 512:(n + 1) * 512], OS[:])
    return nc


_CACHE = {}
LAST_EXEC_NS = {}


def _run(nc, in_maps, **kw):
    import os
    tr = os.environ.get("KTRACE", "") == "1"
    if tr:
        kw.setdefault("trace", True)
    return run_bass_kernel_spmd(nc, in_maps, **kw)


def _get_kernels():
    if "p1" not in _CACHE:
        _CACHE["p1"] = build_phase1()
        _CACHE["p1"].finalize()
        _CACHE["p2"] = build_phase2()
        _CACHE["p2"].finalize()
    return _CACHE["p1"], _CACHE["p2"]


def kernel(x, gamma, beta, W1, b1, W2, b2, w_scale, b_scale):
    x = np.asarray(x, dtype=np.float32)
    W1 = np.asarray(W1, np.float32); b1 = np.asarray(b1, np.float32)
    W2 = np.asarray(W2, np.float32); b2 = np.asarray(b2, np.float32)
    gamma_f = float(np.asarray(gamma).reshape(-1)[0])
    beta_f = float(np.asarray(beta).reshape(-1)[0])
    wsc = float(np.asarray(w_scale).reshape(-1)[0])
    bsc = float(np.asarray(b_scale).reshape(-1)[0])

    nc1, nc2 = _get_kernels()
    xs = np.ascontiguousarray(x.reshape(B, XC))
    wbase = np.tile(np.arange(1, 11, dtype=np.float32), NF * NW)  # [330]
    wfull = np.tile(wbase, (PT, S))
    ident = np.eye(PT, dtype=np.float32)

    in1 = [{"x_in": xs[c * BS:(c + 1) * BS], "wfull_in": wfull, "id_in": ident}
           for c in range(NCORES)]
    r1 = _run(nc1, in1, core_ids=list(range(NCORES)))
    LAST_EXEC_NS["p1"] = r1.exec_time_ns
    rawts = [r["rawt_out"] for r in r1.results]
    # per-column sums / squared sums (col id = cc*128 + p) -> group sums
    csq = np.zeros(RC, np.float64)
    clin = np.zeros(RC, np.float64)
    for r in r1.results:
        csq += r["psq_out"].astype(np.float64).sum(axis=1).T.reshape(-1)
        clin += r["psl_out"].astype(np.float64).sum(axis=1).T.reshape(-1)
    P = np.zeros(64, np.float64)
    for g in range(28):
        P[g if g < 7 else 21 + g] = csq[COLGRP == g].sum()
        if g >= 7:
            P[g] = clin[COLGRP == g].sum()

    # base group BN affines
    A_base = np.zeros(7); C_base = np.zeros(7)
    for g in range(7):
        alpha, bet = GAB[g]
        N = float(B * GCNT[g] * 3)
        S1 = P[7 + g]          # sum of raw (= sum of wsum over group)
        S2 = P[g]              # sum of raw^2
        mT = (alpha * S1 + bet * N) / N
        e2 = (alpha * alpha * S2 + 2 * alpha * bet * S1 + bet * bet * N) / N
        v = e2 - mT * mT
        a = gamma_f / np.sqrt(v + EPS)
        c = beta_f - a * mT
        A_base[g] = a * alpha
        C_base[g] = a * bet + c

    # second-level BN affines (wsum/3, wmax, wmin; p1 = groups 0..5, p2 = {6})
    A2 = np.zeros((3, 7)); C2 = np.zeros((3, 7))
    for si in range(3):
        k = A_base * (1.0 / 3.0 if si == 0 else 1.0)
        off = C_base
        S1g = P[7 + 7 * si:14 + 7 * si].copy()
        S2g = P[28 + 7 * si:35 + 7 * si].copy()
        for grp_set, idxs in (("p1", range(6)), ("p2", [6])):
            Ntot = float(B * sum(GCNT[i] for i in idxs))
            m = sum(k[i] * S1g[i] + B * GCNT[i] * off[i] for i in idxs) / Ntot
            e2 = sum(k[i] ** 2 * S2g[i] + 2 * k[i] * off[i] * S1g[i]
                     + B * GCNT[i] * off[i] ** 2 for i in idxs) / Ntot
            v = e2 - m * m
            a2 = gamma_f / np.sqrt(v + EPS)
            c2 = beta_f - a2 * m
            for i in idxs:
                A2[si, i] = a2 * k[i]
                C2[si, i] = a2 * off[i] + c2

    # per-column affine over the 990 raw columns
    A = np.zeros(990); C = np.zeros(990)
    A[0:495] = np.repeat(A_base[gof], 3); C[0:495] = np.repeat(C_base[gof], 3)
    for si in range(3):
        A[495 + 165 * si:660 + 165 * si] = A2[si, gof]
        C[495 + 165 * si:660 + 165 * si] = C2[si, gof]

    W1e = W1[:, COLMAP]
    W1A = np.zeros((32, RC), np.float32)
    W1A[:30, :990] = W1e * A[None, :].astype(np.float32)
    b1p = np.zeros((32, 1), np.float32)
    b1p[:30, 0] = b1 + W1e @ C.astype(np.float32)
    u = np.zeros((32, 1), np.float32)
    u[:30, 0] = wsc * W2[0]
    c0 = np.float32(wsc * float(b2[0]) + bsc)

    in2 = [{"rawt_in": rawts[c], "w1t_in": np.ascontiguousarray(W1A.T),
            "b1_in": b1p, "u_in": u, "c0_in": np.array([[c0]], np.float32)}
           for c in range(NCORES)]
    r2 = _run(nc2, in2, core_ids=list(range(NCORES)))
    LAST_EXEC_NS["p2"] = r2.exec_time_ns
    # sample id within core = t*1024 + s*128 + p == linear index (identity)
    y = np.concatenate([r["y_out"][0] for r in r2.results])
    return y.astype(np.float32)


# revision 44
# speedup vs baseline: 1.2915x; 1.0226x over previous
"""AlphaNet-v1 Trainium2 kernel: windowed stats + global BatchNorm + tiny MLP.

Strategy (data-parallel over batch, 8 cores):
  Phase 1 (device): per-sample raw features (corr/cov/std/z/dec/mean/ret over
    3 windows of 10 days, plus window sum/max/min) computed sample-major on
    DVE, then PE-transposed to feature-major rawT[1024, BS] in DRAM, plus
    per-column sum-of-squares (ScalarE accum on transposed tiles) and
    per-group linear sums for the BatchNorm statistics.
  Host: combine partial sums -> per-column affine (A, C) for all 990 flat
    features (BN is a global affine; gamma=1 so it commutes with max/min),
    fold into W1' = W1*A, b1' = b1 + W1@C.
  Phase 2 (device): h = relu(W1' @ rawT + b1') via chunked PE matmuls with
    features on the contraction axis (no transposes), y = u @ h + c0.
"""

import numpy as np

import concourse.bass as bass
import concourse.bacc as bacc
import concourse.mybir as mybir
from concourse.tile import TileContext
from concourse.bass_utils import run_bass_kernel_spmd

F32 = mybir.dt.float32
ALU = mybir.AluOpType
AX = mybir.AxisListType
AF = mybir.ActivationFunctionType

B = 131072
NCORES = 8
BS = B // NCORES            # 16384 samples per core
S = 8                       # sample-blocks per tile (each 128 samples)
PT = 128
TSAMP = PT * S              # 1024 samples per tile
NT = BS // TSAMP            # 16 tiles
NF, NW, ND = 11, 3, 10
XC = NF * NW * ND           # 330
RC = 896                    # padded raw feature columns (825 used; window-sum
                            # columns are folded into W1 on the host)
NCC = RC // PT              # 7 feature chunks
EPS = 1e-5

# base feature groups in F (165 features x 3 windows): feat ranges
GF = [(0, 55), (55, 110), (110, 121), (121, 132), (132, 143), (143, 154), (154, 165)]
GCNT = np.array([55, 55, 11, 11, 11, 11, 11], dtype=np.int64)
# (alpha, beta): true_feature = alpha*raw + beta
GAB = [(1.0, 0.0), (1.0 / 9.0, 0.0), (1.0 / 3.0, 0.0), (0.3, 0.0),
       (1.0 / 55.0, 0.0), (0.1, 0.0), (1.0, -1.0)]
# feature slices in F: corr, cov, std, z, dec, mean, ret
SL_CORR, SL_COV, SL_STD, SL_Z, SL_DEC, SL_MEAN, SL_RET = GF
# pair-block offsets for shifted products d=1..10 within 55-pair packing
OFFD = np.cumsum([0] + [11 - d for d in range(1, 11)]).tolist()  # OFFD[d-1] = start


def _feat_perm():
    """map my 165-feature index -> reference feature index (triu pair order)."""
    iu, ju = np.triu_indices(NF, k=1)
    ref_of_pair = {(i, j): k for k, (i, j) in enumerate(zip(iu, ju))}
    fmap = np.arange(165)
    for d in range(1, 11):
        for f in range(NF - d):
            mine = OFFD[d - 1] + f
            ref = ref_of_pair[(f, f + d)]
            fmap[mine] = ref           # corr block
            fmap[55 + mine] = 55 + ref  # cov block
    return fmap


FMAP = _feat_perm()
# stored raw cols: [0:495 base (3*fm+w)] [495:660 wmax] [660:825 wmin]
# (window-sum cols are not stored; their W1 weight folds onto the base cols)
COLMAP_BASE = np.zeros(495, dtype=np.int64)
for _fm in range(165):
    for _w in range(3):
        COLMAP_BASE[3 * _fm + _w] = 3 * FMAP[_fm] + _w

# my raw col -> BN group id (base 0..6; wsum 7+g via base sums; wmax 14+g,
# wmin 21+g), -1 for pad
gof = np.concatenate([np.full(GCNT[g], g) for g in range(7)])  # feat->group
COLGRP = np.full(RC, -1, dtype=np.int64)
COLGRP[0:495] = np.repeat(gof, 3)
COLGRP[495:660] = 14 + gof
COLGRP[660:825] = 21 + gof


def build_phase1():
    nc = bacc.Bacc()
    x_in = nc.dram_tensor("x_in", [BS, XC], F32, kind="ExternalInput")
    wfull_in = nc.dram_tensor("wfull_in", [PT, S * XC], F32, kind="ExternalInput")
    id_in = nc.dram_tensor("id_in", [PT, PT], F32, kind="ExternalInput")
    rawt_out = nc.dram_tensor("rawt_out", [RC, BS], F32, kind="ExternalOutput")
    psq_out = nc.dram_tensor("psq_out", [PT, NT, NCC], F32, kind="ExternalOutput")
    psl_out = nc.dram_tensor("psl_out", [PT, NT, NCC], F32, kind="ExternalOutput")
    psw_out = nc.dram_tensor("psw_out", [PT, NT, 7], F32, kind="ExternalOutput")

    # sample id = t*1024 + s*128 + p  (contiguous 128-sample runs per (t,s))
    x_t = x_in.rearrange("(t s p) c -> t p s c", t=NT, s=S, p=PT)
    # iteration order (cp, cc, j) matches the SBUF-side [p, cc, j] layout
    rt = rawt_out.rearrange("(cc cp) (t s j) -> t s cp cc j",
                            cc=NCC, cp=PT, t=NT, s=S)

    with TileContext(nc) as tc:
        with tc.tile_pool(name="cst", bufs=1) as cp, \
             tc.tile_pool(name="io", bufs=2) as iop, \
             tc.tile_pool(name="wk", bufs=1) as wp, \
             tc.tile_pool(name="big", bufs=2) as bp, \
             tc.tile_pool(name="ft", bufs=1) as fp_, \
             tc.tile_pool(name="tp", bufs=2, space="PSUM") as pp:

            wful = cp.tile([PT, S * XC], F32)
            nc.sync.dma_start(wful[:], wfull_in[:, :])
            IDT = cp.tile([PT, PT], F32)
            nc.sync.dma_start(IDT[:], id_in[:, :])
            SCR = cp.tile([PT, S * 55 * NW], F32)
            PSQ = cp.tile([PT, NT, NCC], F32)
            PSL = cp.tile([PT, NT, NCC], F32)
            PSW = cp.tile([PT, NT, 7], F32)
            Xt = []
            X0 = iop.tile([PT, S * XC], F32, tag="X")
            nc.sync.dma_start(X0.rearrange("p (s c) -> p s c", s=S), x_t[0])
            Xt.append(X0)
            for t in range(NT):
                if t + 1 < NT:
                    Xn = iop.tile([PT, S * XC], F32, tag="X")
                    nc.sync.dma_start(Xn.rearrange("p (s c) -> p s c", s=S),
                                      x_t[t + 1])
                    Xt.append(Xn)
                X = Xt[t]
                Xv = X.rearrange("p (s f w d) -> p s f w d", s=S, f=NF, w=NW, d=ND)
                X4 = X.rearrange("p (s f wd) -> p s f wd", s=S, f=NF, wd=NW * ND)

                RAW = fp_.tile([PT, S * RC], F32, tag="RAW")
                RV = RAW.rearrange("p (s c) -> p s c", s=S)
                F3 = RV[:, :, 0:495].rearrange("p s (f w) -> p s f w", w=NW)
                nc.vector.memset(RV[:, :, 825:RC], 0.0)

                mean3 = F3[:, :, SL_MEAN[0]:SL_MEAN[1], :]
                nc.vector.tensor_reduce(mean3, Xv, axis=AX.X, op=ALU.add)

                # squares -> s2 (squares on ScalarE, reduce on DVE)
                SQ = bp.tile([PT, S * XC], F32, tag="BIG")
                nc.scalar.activation(SQ[:], X[:], AF.Square)
                s2t = wp.tile([PT, S, NF, NW], F32, tag="s2t")
                nc.vector.tensor_reduce(
                    s2t[:], SQ.rearrange("p (s f w d) -> p s f w d",
                                         s=S, f=NF, w=NW, d=ND),
                    axis=AX.X, op=ALU.add)

                # shifted pair products d=1..10 -> SS [p,S,55,3]
                SS = wp.tile([PT, S, 55, NW], F32, tag="SS")
                for d in range(1, 11):
                    o = OFFD[d - 1]
                    PD = bp.tile([PT, S, NF - d, NW * ND], F32, tag="BIG")
                    nc.vector.tensor_tensor(PD[:], X4[:, :, 0:NF - d, :],
                                            X4[:, :, d:NF, :], ALU.mult)
                    nc.vector.tensor_reduce(
                        SS[:, :, o:o + NF - d, :],
                        PD.rearrange("p s f (w d) -> p s f w d", w=NW),
                        axis=AX.X, op=ALU.add)

                # scaled mean (0.1*mean') -> pre-scaled pair products
                MSC = wp.tile([PT, S, NF, NW], F32, tag="MSC")
                nc.vector.tensor_scalar_mul(MSC[:], mean3, 0.1)
                MM = wp.tile([PT, S, 55, NW], F32, tag="MM")
                for d in range(1, 11):
                    o = OFFD[d - 1]
                    nc.vector.tensor_tensor(MM[:, :, o:o + NF - d, :],
                                            MSC[:, :, 0:NF - d, :],
                                            F3[:, :, SL_MEAN[0] + d:SL_MEAN[1], :],
                                            ALU.mult)
                # var' = s2 - 0.1*mean'^2 ; cov' = SS - 0.1*MM
                VT = wp.tile([PT, S, NF, NW], F32, tag="VT")
                nc.vector.tensor_tensor(VT[:], MSC[:], mean3, ALU.mult)
                VARP = wp.tile([PT, S, NF, NW], F32, tag="VARP")
                nc.vector.tensor_tensor(VARP[:], s2t[:], VT[:], ALU.subtract)
                covF = F3[:, :, SL_COV[0]:SL_COV[1], :]
                nc.vector.tensor_tensor(covF, SS[:], MM[:], ALU.subtract)

                # dec = sum(x * d)
                DW = bp.tile([PT, S * XC], F32, tag="BIG")
                nc.vector.tensor_tensor(DW[:], X[:], wful[:], ALU.mult)
                nc.vector.tensor_reduce(
                    F3[:, :, SL_DEC[0]:SL_DEC[1], :],
                    DW.rearrange("p (s f w d) -> p s f w d", s=S, f=NF, w=NW, d=ND),
                    axis=AX.X, op=ALU.add)

                # std_raw = sqrt(var'), rstd = 1/std_raw
                stdF = F3[:, :, SL_STD[0]:SL_STD[1], :]
                nc.scalar.sqrt(stdF, VARP[:])
                RSTD = wp.tile([PT, S, NF, NW], F32, tag="RSTD")
                nc.vector.reciprocal(RSTD[:], stdF)

                # corr = cov' * rstd_i * rstd_j
                RR = wp.tile([PT, S, 55, NW], F32, tag="RR")
                for d in range(1, 11):
                    o = OFFD[d - 1]
                    nc.vector.tensor_tensor(RR[:, :, o:o + NF - d, :],
                                            RSTD[:, :, 0:NF - d, :],
                                            RSTD[:, :, d:NF, :], ALU.mult)
                nc.vector.tensor_tensor(F3[:, :, SL_CORR[0]:SL_CORR[1], :],
                                        covF, RR[:], ALU.mult)

                # z = mean' * rstd
                nc.vector.tensor_tensor(F3[:, :, SL_Z[0]:SL_Z[1], :],
                                        mean3, RSTD[:], ALU.mult)

                # ret = x9 / x0
                R0 = wp.tile([PT, S, NF, NW], F32, tag="R0")
                nc.vector.reciprocal(R0[:], Xv[:, :, :, :, 0])
                nc.vector.tensor_tensor(F3[:, :, SL_RET[0]:SL_RET[1], :],
                                        R0[:], Xv[:, :, :, :, 9], ALU.mult)

                # window stats over the 3 windows: sum -> scratch (folded into
                # W1 on host, only its BN stats needed), max/min -> raw cols
                T01 = wp.tile([PT, S, 165], F32, tag="T01")
                WS = wp.tile([PT, S, 165], F32, tag="WS")
                for si, op in enumerate([ALU.add, ALU.max, ALU.min]):
                    dst = WS[:] if si == 0 else RV[:, :, 330 + 165 * si:495 + 165 * si]
                    nc.vector.tensor_tensor(T01[:], F3[:, :, :, 0], F3[:, :, :, 1], op)
                    nc.vector.tensor_tensor(dst, T01[:], F3[:, :, :, 2], op)
                # E[wsum^2] partial sums per group
                for g, (a, b) in enumerate(GF):
                    scr = SCR[:, 0:S * (b - a)].rearrange("p (s f) -> p s f", s=S)
                    nc.scalar.activation(scr, WS[:, :, a:b], AF.Square,
                                         accum_out=PSW[:, t, g:g + 1])

                # transpose RAW -> feature-major, evac, per-column sums, DMA
                FT = fp_.tile([PT, S, NCC, PT], F32, tag="FT")
                for s in range(S):
                    TP = pp.tile([PT, NCC * PT], F32, tag="TP")
                    for cc in range(NCC):
                        nc.tensor.transpose(TP[:, cc * PT:(cc + 1) * PT],
                                            RV[:, s, cc * PT:(cc + 1) * PT], IDT[:])
                    nc.scalar.copy(FT[:, s, :, :].rearrange("p a b -> p (a b)"), TP[:])
                    nc.sync.dma_start(rt[t, s], FT[:, s, :, :])
                for cc in range(NCC):
                    scr = SCR[:, 0:S * PT].rearrange("p (s j) -> p s j", s=S)
                    nc.scalar.activation(scr, FT[:, :, cc, :], AF.Square,
                                         accum_out=PSQ[:, t, cc:cc + 1])
                    nc.scalar.activation(scr, FT[:, :, cc, :], AF.Copy,
                                         accum_out=PSL[:, t, cc:cc + 1])
            nc.sync.dma_start(psq_out[:, :, :], PSQ[:])
            nc.sync.dma_start(psl_out[:, :, :], PSL[:])
            nc.sync.dma_start(psw_out[:, :, :], PSW[:])
    return nc


def build_phase2():
    nc = bacc.Bacc()
    rawt_in = nc.dram_tensor("rawt_in", [RC, BS], F32, kind="ExternalInput")
    w1t_in = nc.dram_tensor("w1t_in", [RC, 32], F32, kind="ExternalInput")
    b1_in = nc.dram_tensor("b1_in", [32, 1], F32, kind="ExternalInput")
    u_in = nc.dram_tensor("u_in", [32, 1], F32, kind="ExternalInput")
    c0_in = nc.dram_tensor("c0_in", [1, 1], F32, kind="ExternalInput")
    y_out = nc.dram_tensor("y_out", [1, BS], F32, kind="ExternalOutput")

    NB = BS // 512  # 32 blocks of 512 samples
    rtb = rawt_in.rearrange("(cc cp) (n j) -> n cp cc j", cc=NCC, cp=PT, n=NB)

    with TileContext(nc) as tc:
        with tc.tile_pool(name="cst", bufs=1) as cp, \
             tc.tile_pool(name="sb", bufs=3) as sp, \
             tc.tile_pool(name="ps", bufs=4, space="PSUM") as pp:
            W1S = cp.tile([PT, NCC * 32], F32)
            W1Sv = W1S.rearrange("p (c m) -> p c m", c=NCC)
            nc.sync.dma_start(W1Sv,
                              w1t_in.rearrange("(c p) m -> p c m", c=NCC, p=PT))
            B1T = cp.tile([32, 1], F32)
            nc.sync.dma_start(B1T[:], b1_in[:, :])
            UT = cp.tile([32, 1], F32)
            nc.sync.dma_start(UT[:], u_in[:, :])
            C0T = cp.tile([1, 1], F32)
            nc.sync.dma_start(C0T[:], c0_in[:, :])

            for n in range(NB):
                RT = sp.tile([PT, NCC, 512], F32, tag="RT")
                nc.gpsimd.dma_start(RT[:], rtb[n])
                HP = pp.tile([32, 512], F32, tag="HP")
                for cc in range(NCC):
                    nc.tensor.matmul(HP[:], W1Sv[:, cc, :], RT[:, cc, :],
                                     start=(cc == 0), stop=(cc == NCC - 1))
                HS = sp.tile([32, 512], F32, tag="HS")
                nc.scalar.activation(HS[:], HP[:], AF.Relu, bias=B1T[:, 0:1], scale=1.0)
                OP = pp.tile([1, 512], F32, tag="OP")
                nc.tensor.matmul(OP[:], UT[:], HS[:], start=True, stop=True)
                OS = sp.tile([1, 512], F32, tag="OS")
                nc.vector.tensor_scalar(OS[:], OP[:], C0T[0:1, 0:1], None, ALU.add)
                nc.gpsimd.dma_start(y_out[0:1, n * 512:(n + 1) * 512], OS[:])
    return nc


_CACHE = {}
LAST_EXEC_NS = {}


def _run(nc, in_maps, **kw):
    import os
    tr = os.environ.get("KTRACE", "") == "1"
    if tr:
        kw.setdefault("trace", True)
    return run_bass_kernel_spmd(nc, in_maps, **kw)


def _get_kernels():
    if "p1" not in _CACHE:
        _CACHE["p1"] = build_phase1()
        _CACHE["p1"].finalize()
        _CACHE["p2"] = build_phase2()
        _CACHE["p2"].finalize()
    return _CACHE["p1"], _CACHE["p2"]


def kernel(x, gamma, beta, W1, b1, W2, b2, w_scale, b_scale):
    x = np.asarray(x, dtype=np.float32)
    W1 = np.asarray(W1, np.float32); b1 = np.asarray(b1, np.float32)
    W2 = np.asarray(W2, np.float32); b2 = np.asarray(b2, np.float32)
    gamma_f = float(np.asarray(gamma).reshape(-1)[0])
    beta_f = float(np.asarray(beta).reshape(-1)[0])
    wsc = float(np.asarray(w_scale).reshape(-1)[0])
    bsc = float(np.asarray(b_scale).reshape(-1)[0])

    nc1, nc2 = _get_kernels()
    xs = np.ascontiguousarray(x.reshape(B, XC))
    wbase = np.tile(np.arange(1, 11, dtype=np.float32), NF * NW)  # [330]
    wfull = np.tile(wbase, (PT, S))
    ident = np.eye(PT, dtype=np.float32)

    in1 = [{"x_in": xs[c * BS:(c + 1) * BS], "wfull_in": wfull, "id_in": ident}
           for c in range(NCORES)]
    r1 = _run(nc1, in1, core_ids=list(range(NCORES)))
    LAST_EXEC_NS["p1"] = r1.exec_time_ns
    rawts = [r["rawt_out"] for r in r1.results]
    # per-column sums / squared sums (col id = cc*128 + p) -> group sums
    csq = np.zeros(RC, np.float64)
    clin = np.zeros(RC, np.float64)
    psw = np.zeros(7, np.float64)
    for r in r1.results:
        csq += r["psq_out"].astype(np.float64).sum(axis=1).T.reshape(-1)
        clin += r["psl_out"].astype(np.float64).sum(axis=1).T.reshape(-1)
        psw += r["psw_out"].astype(np.float64).sum(axis=(0, 1))
    P = np.zeros(64, np.float64)
    for g in range(7):
        P[g] = csq[COLGRP == g].sum()            # base sum(raw^2)
        P[7 + g] = clin[COLGRP == g].sum()       # base sum(raw) = sum(wsum)
        P[14 + g] = clin[COLGRP == 14 + g].sum()  # wmax linear
        P[21 + g] = clin[COLGRP == 21 + g].sum()  # wmin linear
        P[28 + g] = psw[g]                        # wsum squared
        P[35 + g] = csq[COLGRP == 14 + g].sum()   # wmax squared
        P[42 + g] = csq[COLGRP == 21 + g].sum()   # wmin squared

    # base group BN affines
    A_base = np.zeros(7); C_base = np.zeros(7)
    for g in range(7):
        alpha, bet = GAB[g]
        N = float(B * GCNT[g] * 3)
        S1 = P[7 + g]          # sum of raw (= sum of wsum over group)
        S2 = P[g]              # sum of raw^2
        mT = (alpha * S1 + bet * N) / N
        e2 = (alpha * alpha * S2 + 2 * alpha * bet * S1 + bet * bet * N) / N
        v = e2 - mT * mT
        a = gamma_f / np.sqrt(v + EPS)
        c = beta_f - a * mT
        A_base[g] = a * alpha
        C_base[g] = a * bet + c

    # second-level BN affines (wsum/3, wmax, wmin; p1 = groups 0..5, p2 = {6})
    A2 = np.zeros((3, 7)); C2 = np.zeros((3, 7))
    for si in range(3):
        k = A_base * (1.0 / 3.0 if si == 0 else 1.0)
        off = C_base
        S1g = P[7 + 7 * si:14 + 7 * si].copy()
        S2g = P[28 + 7 * si:35 + 7 * si].copy()
        for grp_set, idxs in (("p1", range(6)), ("p2", [6])):
            Ntot = float(B * sum(GCNT[i] for i in idxs))
            m = sum(k[i] * S1g[i] + B * GCNT[i] * off[i] for i in idxs) / Ntot
            e2 = sum(k[i] ** 2 * S2g[i] + 2 * k[i] * off[i] * S1g[i]
                     + B * GCNT[i] * off[i] ** 2 for i in idxs) / Ntot
            v = e2 - m * m
            a2 = gamma_f / np.sqrt(v + EPS)
            c2 = beta_f - a2 * m
            for i in idxs:
                A2[si, i] = a2 * k[i]
                C2[si, i] = a2 * off[i] + c2

    # reference-order affine over the 990 flat columns (group-constant, so the
    # same values apply in my column order)
    A_ref = np.zeros(990); C_ref = np.zeros(990)
    A_ref[0:495] = np.repeat(A_base[gof], 3)
    C_ref[0:495] = np.repeat(C_base[gof], 3)
    for si in range(3):
        A_ref[495 + 165 * si:660 + 165 * si] = A2[si, gof]
        C_ref[495 + 165 * si:660 + 165 * si] = C2[si, gof]

    # W1 fold: base cols carry their own affine plus the wsum-column weight
    W1A = np.zeros((32, RC), np.float32)
    W1base = W1[:, COLMAP_BASE]                 # [30, 495] my order
    W1ws = W1[:, 495 + FMAP]                    # [30, 165] my feature order
    W1A[:30, 0:495] = (W1base * A_ref[None, 0:495]
                       + np.repeat(W1ws * A2[0, gof][None, :], 3, axis=1)
                       ).astype(np.float32)
    W1A[:30, 495:660] = (W1[:, 660 + FMAP] * A2[1, gof][None, :]).astype(np.float32)
    W1A[:30, 660:825] = (W1[:, 825 + FMAP] * A2[2, gof][None, :]).astype(np.float32)
    b1p = np.zeros((32, 1), np.float32)
    b1p[:30, 0] = b1 + (W1 @ C_ref).astype(np.float32)
    u = np.zeros((32, 1), np.float32)
    u[:30, 0] = wsc * W2[0]
    c0 = np.float32(wsc * float(b2[0]) + bsc)

    in2 = [{"rawt_in": rawts[c], "w1t_in": np.ascontiguousarray(W1A.T),
            "b1_in": b1p, "u_in": u, "c0_in": np.array([[c0]], np.float32)}
           for c in range(NCORES)]
    r2 = _run(nc2, in2, core_ids=list(range(NCORES)))
    LAST_EXEC_NS["p2"] = r2.exec_time_ns
    # sample id within core = t*1024 + s*128 + p == linear index (identity)
    y = np.concatenate([r["y_out"][0] for r in r2.results])
    return y.astype(np.float32)


# revision 45
# speedup vs baseline: 1.2941x; 1.0020x over previous
"""AlphaNet-v1 Trainium2 kernel: windowed stats + global BatchNorm + tiny MLP.

Strategy (data-parallel over batch, 8 cores):
  Phase 1 (device): per-sample raw features (corr/cov/std/z/dec/mean/ret over
    3 windows of 10 days, plus window sum/max/min) computed sample-major on
    DVE, then PE-transposed to feature-major rawT[896, BS] in DRAM, plus
    per-column sum-of-squares (ScalarE accum on transposed tiles) and
    per-group linear sums for the BatchNorm statistics.
  Host: combine partial sums -> per-column affine (A, C) for all 990 flat
    features (BN is a global affine; gamma=1 so it commutes with max/min),
    fold into W1' = W1*A, b1' = b1 + W1@C.
  Phase 2 (device): h = relu(W1' @ rawT + b1') via chunked PE matmuls with
    features on the contraction axis (no transposes), y = u @ h + c0.
"""

import numpy as np

import concourse.bass as bass
import concourse.bacc as bacc
import concourse.mybir as mybir
from concourse.tile import TileContext
from concourse.bass_utils import run_bass_kernel_spmd

F32 = mybir.dt.float32
ALU = mybir.AluOpType
AX = mybir.AxisListType
AF = mybir.ActivationFunctionType

B = 131072
NCORES = 8
BS = B // NCORES            # 16384 samples per core
S = 8                       # sample-blocks per tile (each 128 samples)
PT = 128
TSAMP = PT * S              # 1024 samples per tile
NT = BS // TSAMP            # 16 tiles
NF, NW, ND = 11, 3, 10
XC = NF * NW * ND           # 330
RC = 896                    # padded raw feature columns (825 used; window-sum
                            # columns are folded into W1 on the host)
NCC = RC // PT              # 7 feature chunks
EPS = 1e-5

# base feature groups in F (165 features x 3 windows): feat ranges
GF = [(0, 55), (55, 110), (110, 121), (121, 132), (132, 143), (143, 154), (154, 165)]
GCNT = np.array([55, 55, 11, 11, 11, 11, 11], dtype=np.int64)
# (alpha, beta): true_feature = alpha*raw + beta
GAB = [(1.0, 0.0), (1.0 / 9.0, 0.0), (1.0 / 3.0, 0.0), (0.3, 0.0),
       (1.0 / 55.0, 0.0), (0.1, 0.0), (1.0, -1.0)]
# feature slices in F: corr, cov, std, z, dec, mean, ret
SL_CORR, SL_COV, SL_STD, SL_Z, SL_DEC, SL_MEAN, SL_RET = GF
# pair-block offsets for shifted products d=1..10 within 55-pair packing
OFFD = np.cumsum([0] + [11 - d for d in range(1, 11)]).tolist()  # OFFD[d-1] = start


def _feat_perm():
    """map my 165-feature index -> reference feature index (triu pair order)."""
    iu, ju = np.triu_indices(NF, k=1)
    ref_of_pair = {(i, j): k for k, (i, j) in enumerate(zip(iu, ju))}
    fmap = np.arange(165)
    for d in range(1, 11):
        for f in range(NF - d):
            mine = OFFD[d - 1] + f
            ref = ref_of_pair[(f, f + d)]
            fmap[mine] = ref           # corr block
            fmap[55 + mine] = 55 + ref  # cov block
    return fmap


FMAP = _feat_perm()
# stored raw cols: [0:495 base (3*fm+w)] [495:660 wmax] [660:825 wmin]
# (window-sum cols are not stored; their W1 weight folds onto the base cols)
COLMAP_BASE = np.zeros(495, dtype=np.int64)
for _fm in range(165):
    for _w in range(3):
        COLMAP_BASE[3 * _fm + _w] = 3 * FMAP[_fm] + _w

# my raw col -> BN group id (base 0..6; wsum 7+g via base sums; wmax 14+g,
# wmin 21+g), -1 for pad
gof = np.concatenate([np.full(GCNT[g], g) for g in range(7)])  # feat->group
COLGRP = np.full(RC, -1, dtype=np.int64)
COLGRP[0:495] = np.repeat(gof, 3)
COLGRP[495:660] = 14 + gof
COLGRP[660:825] = 21 + gof


def build_phase1():
    nc = bacc.Bacc()
    x_in = nc.dram_tensor("x_in", [BS, XC], F32, kind="ExternalInput")
    wfull_in = nc.dram_tensor("wfull_in", [PT, S * XC], F32, kind="ExternalInput")
    id_in = nc.dram_tensor("id_in", [PT, PT], F32, kind="ExternalInput")
    rawt_out = nc.dram_tensor("rawt_out", [RC, BS], F32, kind="ExternalOutput")
    psq_out = nc.dram_tensor("psq_out", [PT, NT, NCC], F32, kind="ExternalOutput")
    psl_out = nc.dram_tensor("psl_out", [PT, NT, NCC], F32, kind="ExternalOutput")
    psw_out = nc.dram_tensor("psw_out", [PT, NT, 7], F32, kind="ExternalOutput")

    # sample id = t*1024 + s*128 + p  (contiguous 128-sample runs per (t,s))
    x_t = x_in.rearrange("(t s p) c -> t p s c", t=NT, s=S, p=PT)
    # iteration order (cp, cc, j) matches the SBUF-side [p, cc, j] layout
    rt = rawt_out.rearrange("(cc cp) (t s j) -> t s cp cc j",
                            cc=NCC, cp=PT, t=NT, s=S)

    with TileContext(nc) as tc:
        with tc.tile_pool(name="cst", bufs=1) as cp, \
             tc.tile_pool(name="io", bufs=2) as iop, \
             tc.tile_pool(name="wk", bufs=1) as wp, \
             tc.tile_pool(name="big", bufs=2) as bp, \
             tc.tile_pool(name="ft", bufs=1) as fp_, \
             tc.tile_pool(name="tp", bufs=2, space="PSUM") as pp:

            wful = cp.tile([PT, S * XC], F32)
            nc.sync.dma_start(wful[:], wfull_in[:, :])
            IDT = cp.tile([PT, PT], F32)
            nc.sync.dma_start(IDT[:], id_in[:, :])
            SCR = cp.tile([PT, S * 55 * NW], F32)
            PSQ = cp.tile([PT, NT, NCC], F32)
            PSL = cp.tile([PT, NT, NCC], F32)
            PSW = cp.tile([PT, NT, 7], F32)
            Xt = []
            X0 = iop.tile([PT, S * XC], F32, tag="X")
            nc.sync.dma_start(X0.rearrange("p (s c) -> p s c", s=S), x_t[0])
            Xt.append(X0)
            for t in range(NT):
                if t + 1 < NT:
                    Xn = iop.tile([PT, S * XC], F32, tag="X")
                    nc.sync.dma_start(Xn.rearrange("p (s c) -> p s c", s=S),
                                      x_t[t + 1])
                    Xt.append(Xn)
                X = Xt[t]
                Xv = X.rearrange("p (s f w d) -> p s f w d", s=S, f=NF, w=NW, d=ND)
                X4 = X.rearrange("p (s f wd) -> p s f wd", s=S, f=NF, wd=NW * ND)

                RAW = fp_.tile([PT, S * RC], F32, tag="RAW")
                RV = RAW.rearrange("p (s c) -> p s c", s=S)
                F3 = RV[:, :, 0:495].rearrange("p s (f w) -> p s f w", w=NW)
                nc.vector.memset(RV[:, :, 825:RC], 0.0)

                mean3 = F3[:, :, SL_MEAN[0]:SL_MEAN[1], :]
                nc.vector.tensor_reduce(mean3, Xv, axis=AX.X, op=ALU.add)

                # squares -> s2 (squares on ScalarE, reduce on DVE)
                SQ = bp.tile([PT, S * XC], F32, tag="BIG")
                nc.scalar.activation(SQ[:], X[:], AF.Square)
                s2t = wp.tile([PT, S, NF, NW], F32, tag="s2t")
                nc.vector.tensor_reduce(
                    s2t[:], SQ.rearrange("p (s f w d) -> p s f w d",
                                         s=S, f=NF, w=NW, d=ND),
                    axis=AX.X, op=ALU.add)

                # shifted pair products d=1..10 -> SS [p,S,55,3]
                SS = wp.tile([PT, S, 55, NW], F32, tag="SS")
                for d in range(1, 11):
                    o = OFFD[d - 1]
                    PD = bp.tile([PT, S, NF - d, NW * ND], F32, tag="BIG")
                    nc.vector.tensor_tensor(PD[:], X4[:, :, 0:NF - d, :],
                                            X4[:, :, d:NF, :], ALU.mult)
                    nc.vector.tensor_reduce(
                        SS[:, :, o:o + NF - d, :],
                        PD.rearrange("p s f (w d) -> p s f w d", w=NW),
                        axis=AX.X, op=ALU.add)

                # scaled mean (0.1*mean') -> pre-scaled pair products
                MSC = wp.tile([PT, S, NF, NW], F32, tag="MSC")
                nc.vector.tensor_scalar_mul(MSC[:], mean3, 0.1)
                MM = wp.tile([PT, S, 55, NW], F32, tag="MM")
                for d in range(1, 11):
                    o = OFFD[d - 1]
                    nc.vector.tensor_tensor(MM[:, :, o:o + NF - d, :],
                                            MSC[:, :, 0:NF - d, :],
                                            F3[:, :, SL_MEAN[0] + d:SL_MEAN[1], :],
                                            ALU.mult)
                # var' = s2 - 0.1*mean'^2 ; cov' = SS - 0.1*MM
                VT = wp.tile([PT, S, NF, NW], F32, tag="VT")
                nc.vector.tensor_tensor(VT[:], MSC[:], mean3, ALU.mult)
                VARP = wp.tile([PT, S, NF, NW], F32, tag="VARP")
                nc.vector.tensor_tensor(VARP[:], s2t[:], VT[:], ALU.subtract)
                covF = F3[:, :, SL_COV[0]:SL_COV[1], :]
                nc.vector.tensor_tensor(covF, SS[:], MM[:], ALU.subtract)

                # dec = sum(x * d)
                DW = bp.tile([PT, S * XC], F32, tag="BIG")
                nc.vector.tensor_tensor(DW[:], X[:], wful[:], ALU.mult)
                nc.vector.tensor_reduce(
                    F3[:, :, SL_DEC[0]:SL_DEC[1], :],
                    DW.rearrange("p (s f w d) -> p s f w d", s=S, f=NF, w=NW, d=ND),
                    axis=AX.X, op=ALU.add)

                # std_raw = sqrt(var'), rstd = 1/std_raw
                stdF = F3[:, :, SL_STD[0]:SL_STD[1], :]
                nc.scalar.sqrt(stdF, VARP[:])
                RSTD = wp.tile([PT, S, NF, NW], F32, tag="RSTD")
                nc.vector.reciprocal(RSTD[:], stdF)

                # corr = cov' * rstd_i * rstd_j
                RR = wp.tile([PT, S, 55, NW], F32, tag="RR")
                for d in range(1, 11):
                    o = OFFD[d - 1]
                    nc.vector.tensor_tensor(RR[:, :, o:o + NF - d, :],
                                            RSTD[:, :, 0:NF - d, :],
                                            RSTD[:, :, d:NF, :], ALU.mult)
                nc.vector.tensor_tensor(F3[:, :, SL_CORR[0]:SL_CORR[1], :],
                                        covF, RR[:], ALU.mult)

                # z = mean' * rstd
                nc.vector.tensor_tensor(F3[:, :, SL_Z[0]:SL_Z[1], :],
                                        mean3, RSTD[:], ALU.mult)

                # ret = x9 / x0
                R0 = wp.tile([PT, S, NF, NW], F32, tag="R0")
                nc.vector.reciprocal(R0[:], Xv[:, :, :, :, 0])
                nc.vector.tensor_tensor(F3[:, :, SL_RET[0]:SL_RET[1], :],
                                        R0[:], Xv[:, :, :, :, 9], ALU.mult)

                # window stats over the 3 windows: sum -> scratch (folded into
                # W1 on host, only its BN stats needed), max/min -> raw cols
                T01 = wp.tile([PT, S, 165], F32, tag="T01")
                WS = wp.tile([PT, S, 165], F32, tag="WS")
                for si, op in enumerate([ALU.add, ALU.max, ALU.min]):
                    dst = WS[:] if si == 0 else RV[:, :, 330 + 165 * si:495 + 165 * si]
                    nc.vector.tensor_tensor(T01[:], F3[:, :, :, 0], F3[:, :, :, 1], op)
                    nc.vector.tensor_tensor(dst, T01[:], F3[:, :, :, 2], op)
                # E[wsum^2] partial sums per group
                for g, (a, b) in enumerate(GF):
                    scr = SCR[:, 0:S * (b - a)].rearrange("p (s f) -> p s f", s=S)
                    nc.scalar.activation(scr, WS[:, :, a:b], AF.Square,
                                         accum_out=PSW[:, t, g:g + 1])

                # transpose RAW -> feature-major, evac, per-column sums, DMA
                FT = fp_.tile([PT, S, NCC, PT], F32, tag="FT")
                for s in range(S):
                    TP = pp.tile([PT, NCC * PT], F32, tag="TP")
                    for cc in range(NCC):
                        nc.tensor.transpose(TP[:, cc * PT:(cc + 1) * PT],
                                            RV[:, s, cc * PT:(cc + 1) * PT], IDT[:])
                    nc.scalar.copy(FT[:, s, :, :].rearrange("p a b -> p (a b)"), TP[:])
                    nc.sync.dma_start(rt[t, s], FT[:, s, :, :])
                for cc in range(NCC):
                    scr = SCR[:, 0:S * PT].rearrange("p (s j) -> p s j", s=S)
                    nc.scalar.activation(scr, FT[:, :, cc, :], AF.Square,
                                         accum_out=PSQ[:, t, cc:cc + 1])
                    nc.scalar.activation(scr, FT[:, :, cc, :], AF.Copy,
                                         accum_out=PSL[:, t, cc:cc + 1])
            nc.sync.dma_start(psq_out[:, :, :], PSQ[:])
            nc.sync.dma_start(psl_out[:, :, :], PSL[:])
            nc.sync.dma_start(psw_out[:, :, :], PSW[:])
    return nc


def build_phase2():
    nc = bacc.Bacc()
    rawt_in = nc.dram_tensor("rawt_in", [RC, BS], F32, kind="ExternalInput")
    w1t_in = nc.dram_tensor("w1t_in", [RC, 32], F32, kind="ExternalInput")
    b1_in = nc.dram_tensor("b1_in", [32, 1], F32, kind="ExternalInput")
    u_in = nc.dram_tensor("u_in", [32, 1], F32, kind="ExternalInput")
    c0_in = nc.dram_tensor("c0_in", [1, 1], F32, kind="ExternalInput")
    y_out = nc.dram_tensor("y_out", [1, BS], F32, kind="ExternalOutput")

    NB = BS // 512  # 32 blocks of 512 samples
    rtb = rawt_in.rearrange("(cc cp) (n j) -> n cp cc j", cc=NCC, cp=PT, n=NB)

    with TileContext(nc) as tc:
        with tc.tile_pool(name="cst", bufs=1) as cp, \
             tc.tile_pool(name="sb", bufs=3) as sp, \
             tc.tile_pool(name="ps", bufs=4, space="PSUM") as pp:
            W1S = cp.tile([PT, NCC * 32], F32)
            W1Sv = W1S.rearrange("p (c m) -> p c m", c=NCC)
            nc.sync.dma_start(W1Sv,
                              w1t_in.rearrange("(c p) m -> p c m", c=NCC, p=PT))
            B1T = cp.tile([32, 1], F32)
            nc.sync.dma_start(B1T[:], b1_in[:, :])
            UT = cp.tile([32, 1], F32)
            nc.sync.dma_start(UT[:], u_in[:, :])
            C0T = cp.tile([1, 1], F32)
            nc.sync.dma_start(C0T[:], c0_in[:, :])

            for n in range(NB):
                RT = sp.tile([PT, NCC, 512], F32, tag="RT")
                nc.gpsimd.dma_start(RT[:], rtb[n])
                HP = pp.tile([32, 512], F32, tag="HP")
                for cc in range(NCC):
                    nc.tensor.matmul(HP[:], W1Sv[:, cc, :], RT[:, cc, :],
                                     start=(cc == 0), stop=(cc == NCC - 1))
                HS = sp.tile([32, 512], F32, tag="HS")
                nc.scalar.activation(HS[:], HP[:], AF.Relu, bias=B1T[:, 0:1], scale=1.0)
                OP = pp.tile([1, 512], F32, tag="OP")
                nc.tensor.matmul(OP[:], UT[:], HS[:], start=True, stop=True)
                OS = sp.tile([1, 512], F32, tag="OS")
                nc.vector.tensor_scalar(OS[:], OP[:], C0T[0:1, 0:1], None, ALU.add)
                nc.gpsimd.dma_start(y_out[0:1, n * 512:(n + 1) * 512], OS[:])
    return nc


_CACHE = {}
LAST_EXEC_NS = {}


def _run(nc, in_maps, **kw):
    import os
    tr = os.environ.get("KTRACE", "") == "1"
    if tr:
        kw.setdefault("trace", True)
    return run_bass_kernel_spmd(nc, in_maps, **kw)


def _get_kernels():
    if "p1" not in _CACHE:
        _CACHE["p1"] = build_phase1()
        _CACHE["p1"].finalize()
        _CACHE["p2"] = build_phase2()
        _CACHE["p2"].finalize()
    return _CACHE["p1"], _CACHE["p2"]


def kernel(x, gamma, beta, W1, b1, W2, b2, w_scale, b_scale):
    x = np.asarray(x, dtype=np.float32)
    W1 = np.asarray(W1, np.float32); b1 = np.asarray(b1, np.float32)
    W2 = np.asarray(W2, np.float32); b2 = np.asarray(b2, np.float32)
    gamma_f = float(np.asarray(gamma).reshape(-1)[0])
    beta_f = float(np.asarray(beta).reshape(-1)[0])
    wsc = float(np.asarray(w_scale).reshape(-1)[0])
    bsc = float(np.asarray(b_scale).reshape(-1)[0])

    nc1, nc2 = _get_kernels()
    xs = np.ascontiguousarray(x.reshape(B, XC))
    wbase = np.tile(np.arange(1, 11, dtype=np.float32), NF * NW)  # [330]
    wfull = np.tile(wbase, (PT, S))
    ident = np.eye(PT, dtype=np.float32)

    in1 = [{"x_in": xs[c * BS:(c + 1) * BS], "wfull_in": wfull, "id_in": ident}
           for c in range(NCORES)]
    r1 = _run(nc1, in1, core_ids=list(range(NCORES)))
    LAST_EXEC_NS["p1"] = r1.exec_time_ns
    rawts = [r["rawt_out"] for r in r1.results]
    # per-column sums / squared sums (col id = cc*128 + p) -> group sums
    csq = np.zeros(RC, np.float64)
    clin = np.zeros(RC, np.float64)
    psw = np.zeros(7, np.float64)
    for r in r1.results:
        csq += r["psq_out"].astype(np.float64).sum(axis=1).T.reshape(-1)
        clin += r["psl_out"].astype(np.float64).sum(axis=1).T.reshape(-1)
        psw += r["psw_out"].astype(np.float64).sum(axis=(0, 1))
    P = np.zeros(64, np.float64)
    for g in range(7):
        P[g] = csq[COLGRP == g].sum()            # base sum(raw^2)
        P[7 + g] = clin[COLGRP == g].sum()       # base sum(raw) = sum(wsum)
        P[14 + g] = clin[COLGRP == 14 + g].sum()  # wmax linear
        P[21 + g] = clin[COLGRP == 21 + g].sum()  # wmin linear
        P[28 + g] = psw[g]                        # wsum squared
        P[35 + g] = csq[COLGRP == 14 + g].sum()   # wmax squared
        P[42 + g] = csq[COLGRP == 21 + g].sum()   # wmin squared

    # base group BN affines
    A_base = np.zeros(7); C_base = np.zeros(7)
    for g in range(7):
        alpha, bet = GAB[g]
        N = float(B * GCNT[g] * 3)
        S1 = P[7 + g]          # sum of raw (= sum of wsum over group)
        S2 = P[g]              # sum of raw^2
        mT = (alpha * S1 + bet * N) / N
        e2 = (alpha * alpha * S2 + 2 * alpha * bet * S1 + bet * bet * N) / N
        v = e2 - mT * mT
        a = gamma_f / np.sqrt(v + EPS)
        c = beta_f - a * mT
        A_base[g] = a * alpha
        C_base[g] = a * bet + c

    # second-level BN affines (wsum/3, wmax, wmin; p1 = groups 0..5, p2 = {6})
    A2 = np.zeros((3, 7)); C2 = np.zeros((3, 7))
    for si in range(3):
        k = A_base * (1.0 / 3.0 if si == 0 else 1.0)
        off = C_base
        S1g = P[7 + 7 * si:14 + 7 * si].copy()
        S2g = P[28 + 7 * si:35 + 7 * si].copy()
        for grp_set, idxs in (("p1", range(6)), ("p2", [6])):
            Ntot = float(B * sum(GCNT[i] for i in idxs))
            m = sum(k[i] * S1g[i] + B * GCNT[i] * off[i] for i in idxs) / Ntot
            e2 = sum(k[i] ** 2 * S2g[i] + 2 * k[i] * off[i] * S1g[i]
                     + B * GCNT[i] * off[i] ** 2 for i in idxs) / Ntot
            v = e2 - m * m
            a2 = gamma_f / np.sqrt(v + EPS)
            c2 = beta_f - a2 * m
            for i in idxs:
                A2[si, i] = a2 * k[i]
                C2[si, i] = a2 * off[i] + c2

    # reference-order affine over the 990 flat columns (group-constant, so the
    # same values apply in my column order)
    A_ref = np.zeros(990); C_ref = np.zeros(990)
    A_ref[0:495] = np.repeat(A_base[gof], 3)
    C_ref[0:495] = np.repeat(C_base[gof], 3)
    for si in range(3):
        A_ref[495 + 165 * si:660 + 165 * si] = A2[si, gof]
        C_ref[495 + 165 * si:660 + 165 * si] = C2[si, gof]

    # W1 fold: base cols carry their own affine plus the wsum-column weight
    W1A = np.zeros((32, RC), np.float32)
    W1base = W1[:, COLMAP_BASE]                 # [30, 495] my order
    W1ws = W1[:, 495 + FMAP]                    # [30, 165] my feature order
    W1A[:30, 0:495] = (W1base * A_ref[None, 0:495]
                       + np.repeat(W1ws * A2[0, gof][None, :], 3, axis=1)
                       ).astype(np.float32)
    W1A[:30, 495:660] = (W1[:, 660 + FMAP] * A2[1, gof][None, :]).astype(np.float32)
    W1A[:30, 660:825] = (W1[:, 825 + FMAP] * A2[2, gof][None, :]).astype(np.float32)
    b1p = np.zeros((32, 1), np.float32)
    b1p[:30, 0] = b1 + (W1 @ C_ref).astype(np.float32)
    u = np.zeros((32, 1), np.float32)
    u[:30, 0] = wsc * W2[0]
    c0 = np.float32(wsc * float(b2[0]) + bsc)

    in2 = [{"rawt_in": rawts[c], "w1t_in": np.ascontiguousarray(W1A.T),
            "b1_in": b1p, "u_in": u, "c0_in": np.array([[c0]], np.float32)}
           for c in range(NCORES)]
    r2 = _run(nc2, in2, core_ids=list(range(NCORES)))
    LAST_EXEC_NS["p2"] = r2.exec_time_ns
    # sample id within core = t*1024 + s*128 + p == linear index (identity)
    y = np.concatenate([r["y_out"][0] for r in r2.results])
    return y.astype(np.float32)


# revision 47
# speedup vs baseline: 1.3117x; 1.0136x over previous
"""AlphaNet-v1 Trainium2 kernel: windowed stats + global BatchNorm + tiny MLP.

Strategy (data-parallel over batch, 8 cores):
  Phase 1 (device): per-sample raw features (corr/cov/std/z/dec/mean/ret over
    3 windows of 10 days, plus window sum/max/min) computed sample-major on
    DVE, then PE-transposed to feature-major rawT[896, BS] in DRAM, plus
    per-column sum-of-squares (ScalarE accum on transposed tiles) and
    per-group linear sums for the BatchNorm statistics.
  Host: combine partial sums -> per-column affine (A, C) for all 990 flat
    features (BN is a global affine; gamma=1 so it commutes with max/min),
    fold into W1' = W1*A, b1' = b1 + W1@C.
  Phase 2 (device): h = relu(W1' @ rawT + b1') via chunked PE matmuls with
    features on the contraction axis (no transposes), y = u @ h + c0.
"""

import numpy as np

import concourse.bass as bass
import concourse.bacc as bacc
import concourse.mybir as mybir
from concourse.tile import TileContext
from concourse.bass_utils import run_bass_kernel_spmd

F32 = mybir.dt.float32
ALU = mybir.AluOpType
AX = mybir.AxisListType
AF = mybir.ActivationFunctionType

B = 131072
NCORES = 8
BS = B // NCORES            # 16384 samples per core
S = 8                       # sample-blocks per tile (each 128 samples)
PT = 128
TSAMP = PT * S              # 1024 samples per tile
NT = BS // TSAMP            # 16 tiles
NF, NW, ND = 11, 3, 10
XC = NF * NW * ND           # 330
RC = 896                    # padded raw feature columns (825 used; window-sum
                            # columns are folded into W1 on the host)
NCC = RC // PT              # 7 feature chunks
EPS = 1e-5

# base feature groups in F (165 features x 3 windows): feat ranges
GF = [(0, 55), (55, 110), (110, 121), (121, 132), (132, 143), (143, 154), (154, 165)]
GCNT = np.array([55, 55, 11, 11, 11, 11, 11], dtype=np.int64)
# (alpha, beta): true_feature = alpha*raw + beta
GAB = [(1.0, 0.0), (1.0 / 9.0, 0.0), (1.0 / 3.0, 0.0), (0.3, 0.0),
       (1.0 / 55.0, 0.0), (0.1, 0.0), (1.0, -1.0)]
# feature slices in F: corr, cov, std, z, dec, mean, ret
SL_CORR, SL_COV, SL_STD, SL_Z, SL_DEC, SL_MEAN, SL_RET = GF
# pair-block offsets for shifted products d=1..10 within 55-pair packing
OFFD = np.cumsum([0] + [11 - d for d in range(1, 11)]).tolist()  # OFFD[d-1] = start


def _feat_perm():
    """map my 165-feature index -> reference feature index (triu pair order)."""
    iu, ju = np.triu_indices(NF, k=1)
    ref_of_pair = {(i, j): k for k, (i, j) in enumerate(zip(iu, ju))}
    fmap = np.arange(165)
    for d in range(1, 11):
        for f in range(NF - d):
            mine = OFFD[d - 1] + f
            ref = ref_of_pair[(f, f + d)]
            fmap[mine] = ref           # corr block
            fmap[55 + mine] = 55 + ref  # cov block
    return fmap


FMAP = _feat_perm()
# stored raw cols: [0:495 base (3*fm+w)] [495:660 wmax] [660:825 wmin]
# (window-sum cols are not stored; their W1 weight folds onto the base cols)
COLMAP_BASE = np.zeros(495, dtype=np.int64)
for _fm in range(165):
    for _w in range(3):
        COLMAP_BASE[3 * _fm + _w] = 3 * FMAP[_fm] + _w

# my raw col -> BN group id (base 0..6; wsum 7+g via base sums; wmax 14+g,
# wmin 21+g), -1 for pad
gof = np.concatenate([np.full(GCNT[g], g) for g in range(7)])  # feat->group
COLGRP = np.full(RC, -1, dtype=np.int64)
COLGRP[0:495] = np.repeat(gof, 3)
COLGRP[495:660] = 14 + gof
COLGRP[660:825] = 21 + gof


def build_phase1():
    nc = bacc.Bacc()
    x_in = nc.dram_tensor("x_in", [BS, XC], F32, kind="ExternalInput")
    wfull_in = nc.dram_tensor("wfull_in", [PT, S * XC], F32, kind="ExternalInput")
    id_in = nc.dram_tensor("id_in", [PT, PT], F32, kind="ExternalInput")
    rawt_out = nc.dram_tensor("rawt_out", [RC, BS], F32, kind="ExternalOutput")
    psq_out = nc.dram_tensor("psq_out", [PT, NT, NCC], F32, kind="ExternalOutput")
    psl_out = nc.dram_tensor("psl_out", [PT, NT, NCC], F32, kind="ExternalOutput")
    psw_out = nc.dram_tensor("psw_out", [PT, NT, 7], F32, kind="ExternalOutput")

    # sample id = t*1024 + s*128 + p  (contiguous 128-sample runs per (t,s))
    x_t = x_in.rearrange("(t s p) c -> t p s c", t=NT, s=S, p=PT)
    # iteration order (cp, cc, j) matches the SBUF-side [p, cc, j] layout
    rt = rawt_out.rearrange("(cc cp) (t s j) -> t s cp cc j",
                            cc=NCC, cp=PT, t=NT, s=S)

    with TileContext(nc) as tc:
        with tc.tile_pool(name="cst", bufs=1) as cp, \
             tc.tile_pool(name="io", bufs=2) as iop, \
             tc.tile_pool(name="wk", bufs=1) as wp, \
             tc.tile_pool(name="big", bufs=2) as bp, \
             tc.tile_pool(name="ft", bufs=2) as fp_, \
             tc.tile_pool(name="tp", bufs=2, space="PSUM") as pp:

            wful = cp.tile([PT, S * XC], F32)
            nc.sync.dma_start(wful[:], wfull_in[:, :])
            IDT = cp.tile([PT, PT], F32)
            nc.sync.dma_start(IDT[:], id_in[:, :])
            SCR = cp.tile([PT, S * 55 * NW], F32)
            PSQ = cp.tile([PT, NT, NCC], F32)
            PSL = cp.tile([PT, NT, NCC], F32)
            PSW = cp.tile([PT, NT, 7], F32)
            Xt = []
            X0 = iop.tile([PT, S * XC], F32, tag="X")
            nc.sync.dma_start(X0.rearrange("p (s c) -> p s c", s=S), x_t[0])
            Xt.append(X0)
            for t in range(NT):
                if t + 1 < NT:
                    Xn = iop.tile([PT, S * XC], F32, tag="X")
                    nc.sync.dma_start(Xn.rearrange("p (s c) -> p s c", s=S),
                                      x_t[t + 1])
                    Xt.append(Xn)
                X = Xt[t]
                Xv = X.rearrange("p (s f w d) -> p s f w d", s=S, f=NF, w=NW, d=ND)
                X4 = X.rearrange("p (s f wd) -> p s f wd", s=S, f=NF, wd=NW * ND)

                RAW = fp_.tile([PT, S * RC], F32, tag="RAW")
                RV = RAW.rearrange("p (s c) -> p s c", s=S)
                F3 = RV[:, :, 0:495].rearrange("p s (f w) -> p s f w", w=NW)
                nc.vector.memset(RV[:, :, 825:RC], 0.0)

                mean3 = F3[:, :, SL_MEAN[0]:SL_MEAN[1], :]
                nc.vector.tensor_reduce(mean3, Xv, axis=AX.X, op=ALU.add)

                # squares -> s2 (squares on ScalarE, reduce on DVE)
                SQ = bp.tile([PT, S * XC], F32, tag="BIG")
                nc.scalar.activation(SQ[:], X[:], AF.Square)
                s2t = wp.tile([PT, S, NF, NW], F32, tag="s2t")
                nc.vector.tensor_reduce(
                    s2t[:], SQ.rearrange("p (s f w d) -> p s f w d",
                                         s=S, f=NF, w=NW, d=ND),
                    axis=AX.X, op=ALU.add)

                # shifted pair products d=1..10 -> SS [p,S,55,3]
                SS = wp.tile([PT, S, 55, NW], F32, tag="SS")
                for d in range(1, 11):
                    o = OFFD[d - 1]
                    PD = bp.tile([PT, S, NF - d, NW * ND], F32, tag="BIG")
                    nc.vector.tensor_tensor(PD[:], X4[:, :, 0:NF - d, :],
                                            X4[:, :, d:NF, :], ALU.mult)
                    nc.vector.tensor_reduce(
                        SS[:, :, o:o + NF - d, :],
                        PD.rearrange("p s f (w d) -> p s f w d", w=NW),
                        axis=AX.X, op=ALU.add)

                # scaled mean (0.1*mean') -> pre-scaled pair products
                MSC = wp.tile([PT, S, NF, NW], F32, tag="MSC")
                nc.vector.tensor_scalar_mul(MSC[:], mean3, 0.1)
                MM = wp.tile([PT, S, 55, NW], F32, tag="MM")
                for d in range(1, 11):
                    o = OFFD[d - 1]
                    nc.vector.tensor_tensor(MM[:, :, o:o + NF - d, :],
                                            MSC[:, :, 0:NF - d, :],
                                            F3[:, :, SL_MEAN[0] + d:SL_MEAN[1], :],
                                            ALU.mult)
                # var' = s2 - 0.1*mean'^2 ; cov' = SS - 0.1*MM
                VT = wp.tile([PT, S, NF, NW], F32, tag="VT")
                nc.vector.tensor_tensor(VT[:], MSC[:], mean3, ALU.mult)
                VARP = wp.tile([PT, S, NF, NW], F32, tag="VARP")
                nc.vector.tensor_tensor(VARP[:], s2t[:], VT[:], ALU.subtract)
                covF = F3[:, :, SL_COV[0]:SL_COV[1], :]
                nc.vector.tensor_tensor(covF, SS[:], MM[:], ALU.subtract)

                # dec = sum(x * d)
                DW = bp.tile([PT, S * XC], F32, tag="BIG")
                nc.vector.tensor_tensor(DW[:], X[:], wful[:], ALU.mult)
                nc.vector.tensor_reduce(
                    F3[:, :, SL_DEC[0]:SL_DEC[1], :],
                    DW.rearrange("p (s f w d) -> p s f w d", s=S, f=NF, w=NW, d=ND),
                    axis=AX.X, op=ALU.add)

                # std_raw = sqrt(var'), rstd = 1/std_raw
                stdF = F3[:, :, SL_STD[0]:SL_STD[1], :]
                nc.scalar.sqrt(stdF, VARP[:])
                RSTD = wp.tile([PT, S, NF, NW], F32, tag="RSTD")
                nc.vector.reciprocal(RSTD[:], stdF)

                # corr = cov' * rstd_i * rstd_j
                RR = wp.tile([PT, S, 55, NW], F32, tag="RR")
                for d in range(1, 11):
                    o = OFFD[d - 1]
                    nc.vector.tensor_tensor(RR[:, :, o:o + NF - d, :],
                                            RSTD[:, :, 0:NF - d, :],
                                            RSTD[:, :, d:NF, :], ALU.mult)
                nc.vector.tensor_tensor(F3[:, :, SL_CORR[0]:SL_CORR[1], :],
                                        covF, RR[:], ALU.mult)

                # z = mean' * rstd
                nc.vector.tensor_tensor(F3[:, :, SL_Z[0]:SL_Z[1], :],
                                        mean3, RSTD[:], ALU.mult)

                # ret = x9 / x0
                R0 = wp.tile([PT, S, NF, NW], F32, tag="R0")
                nc.vector.reciprocal(R0[:], Xv[:, :, :, :, 0])
                nc.vector.tensor_tensor(F3[:, :, SL_RET[0]:SL_RET[1], :],
                                        R0[:], Xv[:, :, :, :, 9], ALU.mult)

                # window stats over the 3 windows: sum -> scratch (folded into
                # W1 on host, only its BN stats needed), max/min -> raw cols
                T01 = wp.tile([PT, S, 165], F32, tag="T01")
                WS = wp.tile([PT, S, 165], F32, tag="WS")
                for si, op in enumerate([ALU.add, ALU.max, ALU.min]):
                    dst = WS[:] if si == 0 else RV[:, :, 330 + 165 * si:495 + 165 * si]
                    nc.vector.tensor_tensor(T01[:], F3[:, :, :, 0], F3[:, :, :, 1], op)
                    nc.vector.tensor_tensor(dst, T01[:], F3[:, :, :, 2], op)
                # E[wsum^2] partial sums per group
                for g, (a, b) in enumerate(GF):
                    scr = SCR[:, 0:S * (b - a)].rearrange("p (s f) -> p s f", s=S)
                    nc.scalar.activation(scr, WS[:, :, a:b], AF.Square,
                                         accum_out=PSW[:, t, g:g + 1])

                # transpose RAW -> feature-major IN PLACE (the evacuated
                # transpose of slot s overwrites slot s's raw cols — same
                # 896 floats), then per-column sums + DMA out
                for s in range(S):
                    TP = pp.tile([PT, NCC * PT], F32, tag="TP")
                    for cc in range(NCC):
                        nc.tensor.transpose(TP[:, cc * PT:(cc + 1) * PT],
                                            RV[:, s, cc * PT:(cc + 1) * PT], IDT[:])
                    nc.scalar.copy(RV[:, s, :], TP[:])
                    nc.sync.dma_start(
                        rt[t, s],
                        RV[:, s, :].rearrange("p (cc j) -> p cc j", cc=NCC))
                for cc in range(NCC):
                    scr = SCR[:, 0:S * PT].rearrange("p (s j) -> p s j", s=S)
                    nc.scalar.activation(scr, RV[:, :, cc * PT:(cc + 1) * PT],
                                         AF.Square,
                                         accum_out=PSQ[:, t, cc:cc + 1])
                    nc.scalar.activation(scr, RV[:, :, cc * PT:(cc + 1) * PT],
                                         AF.Copy,
                                         accum_out=PSL[:, t, cc:cc + 1])
            nc.sync.dma_start(psq_out[:, :, :], PSQ[:])
            nc.sync.dma_start(psl_out[:, :, :], PSL[:])
            nc.sync.dma_start(psw_out[:, :, :], PSW[:])
    return nc


def build_phase2():
    nc = bacc.Bacc()
    rawt_in = nc.dram_tensor("rawt_in", [RC, BS], F32, kind="ExternalInput")
    w1t_in = nc.dram_tensor("w1t_in", [RC, 32], F32, kind="ExternalInput")
    b1_in = nc.dram_tensor("b1_in", [32, 1], F32, kind="ExternalInput")
    u_in = nc.dram_tensor("u_in", [32, 1], F32, kind="ExternalInput")
    c0_in = nc.dram_tensor("c0_in", [1, 1], F32, kind="ExternalInput")
    y_out = nc.dram_tensor("y_out", [1, BS], F32, kind="ExternalOutput")

    NB = BS // 512  # 32 blocks of 512 samples
    rtb = rawt_in.rearrange("(cc cp) (n j) -> n cp cc j", cc=NCC, cp=PT, n=NB)

    with TileContext(nc) as tc:
        with tc.tile_pool(name="cst", bufs=1) as cp, \
             tc.tile_pool(name="sb", bufs=3) as sp, \
             tc.tile_pool(name="ps", bufs=4, space="PSUM") as pp:
            W1S = cp.tile([PT, NCC * 32], F32)
            W1Sv = W1S.rearrange("p (c m) -> p c m", c=NCC)
            nc.sync.dma_start(W1Sv,
                              w1t_in.rearrange("(c p) m -> p c m", c=NCC, p=PT))
            B1T = cp.tile([32, 1], F32)
            nc.sync.dma_start(B1T[:], b1_in[:, :])
            UT = cp.tile([32, 1], F32)
            nc.sync.dma_start(UT[:], u_in[:, :])
            C0T = cp.tile([1, 1], F32)
            nc.sync.dma_start(C0T[:], c0_in[:, :])

            for n in range(NB):
                RT = sp.tile([PT, NCC, 512], F32, tag="RT")
                nc.gpsimd.dma_start(RT[:], rtb[n])
                HP = pp.tile([32, 512], F32, tag="HP")
                for cc in range(NCC):
                    nc.tensor.matmul(HP[:], W1Sv[:, cc, :], RT[:, cc, :],
                                     start=(cc == 0), stop=(cc == NCC - 1))
                HS = sp.tile([32, 512], F32, tag="HS")
                nc.scalar.activation(HS[:], HP[:], AF.Relu, bias=B1T[:, 0:1], scale=1.0)
                OP = pp.tile([1, 512], F32, tag="OP")
                nc.tensor.matmul(OP[:], UT[:], HS[:], start=True, stop=True)
                OS = sp.tile([1, 512], F32, tag="OS")
                nc.vector.tensor_scalar(OS[:], OP[:], C0T[0:1, 0:1], None, ALU.add)
                nc.gpsimd.dma_start(y_out[0:1, n * 512:(n + 1) * 512], OS[:])
    return nc


_CACHE = {}
LAST_EXEC_NS = {}


def _run(nc, in_maps, **kw):
    import os
    tr = os.environ.get("KTRACE", "") == "1"
    if tr:
        kw.setdefault("trace", True)
    return run_bass_kernel_spmd(nc, in_maps, **kw)


def _get_kernels():
    if "p1" not in _CACHE:
        _CACHE["p1"] = build_phase1()
        _CACHE["p1"].finalize()
        _CACHE["p2"] = build_phase2()
        _CACHE["p2"].finalize()
    return _CACHE["p1"], _CACHE["p2"]


def kernel(x, gamma, beta, W1, b1, W2, b2, w_scale, b_scale):
    x = np.asarray(x, dtype=np.float32)
    W1 = np.asarray(W1, np.float32); b1 = np.asarray(b1, np.float32)
    W2 = np.asarray(W2, np.float32); b2 = np.asarray(b2, np.float32)
    gamma_f = float(np.asarray(gamma).reshape(-1)[0])
    beta_f = float(np.asarray(beta).reshape(-1)[0])
    wsc = float(np.asarray(w_scale).reshape(-1)[0])
    bsc = float(np.asarray(b_scale).reshape(-1)[0])

    nc1, nc2 = _get_kernels()
    xs = np.ascontiguousarray(x.reshape(B, XC))
    wbase = np.tile(np.arange(1, 11, dtype=np.float32), NF * NW)  # [330]
    wfull = np.tile(wbase, (PT, S))
    ident = np.eye(PT, dtype=np.float32)

    in1 = [{"x_in": xs[c * BS:(c + 1) * BS], "wfull_in": wfull, "id_in": ident}
           for c in range(NCORES)]
    r1 = _run(nc1, in1, core_ids=list(range(NCORES)))
    LAST_EXEC_NS["p1"] = r1.exec_time_ns
    rawts = [r["rawt_out"] for r in r1.results]
    # per-column sums / squared sums (col id = cc*128 + p) -> group sums
    csq = np.zeros(RC, np.float64)
    clin = np.zeros(RC, np.float64)
    psw = np.zeros(7, np.float64)
    for r in r1.results:
        csq += r["psq_out"].astype(np.float64).sum(axis=1).T.reshape(-1)
        clin += r["psl_out"].astype(np.float64).sum(axis=1).T.reshape(-1)
        psw += r["psw_out"].astype(np.float64).sum(axis=(0, 1))
    P = np.zeros(64, np.float64)
    for g in range(7):
        P[g] = csq[COLGRP == g].sum()            # base sum(raw^2)
        P[7 + g] = clin[COLGRP == g].sum()       # base sum(raw) = sum(wsum)
        P[14 + g] = clin[COLGRP == 14 + g].sum()  # wmax linear
        P[21 + g] = clin[COLGRP == 21 + g].sum()  # wmin linear
        P[28 + g] = psw[g]                        # wsum squared
        P[35 + g] = csq[COLGRP == 14 + g].sum()   # wmax squared
        P[42 + g] = csq[COLGRP == 21 + g].sum()   # wmin squared

    # base group BN affines
    A_base = np.zeros(7); C_base = np.zeros(7)
    for g in range(7):
        alpha, bet = GAB[g]
        N = float(B * GCNT[g] * 3)
        S1 = P[7 + g]          # sum of raw (= sum of wsum over group)
        S2 = P[g]              # sum of raw^2
        mT = (alpha * S1 + bet * N) / N
        e2 = (alpha * alpha * S2 + 2 * alpha * bet * S1 + bet * bet * N) / N
        v = e2 - mT * mT
        a = gamma_f / np.sqrt(v + EPS)
        c = beta_f - a * mT
        A_base[g] = a * alpha
        C_base[g] = a * bet + c

    # second-level BN affines (wsum/3, wmax, wmin; p1 = groups 0..5, p2 = {6})
    A2 = np.zeros((3, 7)); C2 = np.zeros((3, 7))
    for si in range(3):
        k = A_base * (1.0 / 3.0 if si == 0 else 1.0)
        off = C_base
        S1g = P[7 + 7 * si:14 + 7 * si].copy()
        S2g = P[28 + 7 * si:35 + 7 * si].copy()
        for grp_set, idxs in (("p1", range(6)), ("p2", [6])):
            Ntot = float(B * sum(GCNT[i] for i in idxs))
            m = sum(k[i] * S1g[i] + B * GCNT[i] * off[i] for i in idxs) / Ntot
            e2 = sum(k[i] ** 2 * S2g[i] + 2 * k[i] * off[i] * S1g[i]
                     + B * GCNT[i] * off[i] ** 2 for i in idxs) / Ntot
            v = e2 - m * m
            a2 = gamma_f / np.sqrt(v + EPS)
            c2 = beta_f - a2 * m
            for i in idxs:
                A2[si, i] = a2 * k[i]
                C2[si, i] = a2 * off[i] + c2

    # reference-order affine over the 990 flat columns (group-constant, so the
    # same values apply in my column order)
    A_ref = np.zeros(990); C_ref = np.zeros(990)
    A_ref[0:495] = np.repeat(A_base[gof], 3)
    C_ref[0:495] = np.repeat(C_base[gof], 3)
    for si in range(3):
        A_ref[495 + 165 * si:660 + 165 * si] = A2[si, gof]
        C_ref[495 + 165 * si:660 + 165 * si] = C2[si, gof]

    # W1 fold: base cols carry their own affine plus the wsum-column weight
    W1A = np.zeros((32, RC), np.float32)
    W1base = W1[:, COLMAP_BASE]                 # [30, 495] my order
    W1ws = W1[:, 495 + FMAP]                    # [30, 165] my feature order
    W1A[:30, 0:495] = (W1base * A_ref[None, 0:495]
                       + np.repeat(W1ws * A2[0, gof][None, :], 3, axis=1)
                       ).astype(np.float32)
    W1A[:30, 495:660] = (W1[:, 660 + FMAP] * A2[1, gof][None, :]).astype(np.float32)
    W1A[:30, 660:825] = (W1[:, 825 + FMAP] * A2[2, gof][None, :]).astype(np.float32)
    b1p = np.zeros((32, 1), np.float32)
    b1p[:30, 0] = b1 + (W1 @ C_ref).astype(np.float32)
    u = np.zeros((32, 1), np.float32)
    u[:30, 0] = wsc * W2[0]
    c0 = np.float32(wsc * float(b2[0]) + bsc)

    in2 = [{"rawt_in": rawts[c], "w1t_in": np.ascontiguousarray(W1A.T),
            "b1_in": b1p, "u_in": u, "c0_in": np.array([[c0]], np.float32)}
           for c in range(NCORES)]
    r2 = _run(nc2, in2, core_ids=list(range(NCORES)))
    LAST_EXEC_NS["p2"] = r2.exec_time_ns
    # sample id within core = t*1024 + s*128 + p == linear index (identity)
    y = np.concatenate([r["y_out"][0] for r in r2.results])
    return y.astype(np.float32)


# revision 51
# speedup vs baseline: 1.3356x; 1.0182x over previous
"""AlphaNet-v1 Trainium2 kernel: windowed stats + global BatchNorm + tiny MLP.

Strategy (data-parallel over batch, 8 cores):
  Phase 1 (device): per-sample raw features (corr/cov/std/z/dec/mean/ret over
    3 windows of 10 days, plus window sum/max/min) computed sample-major on
    DVE, then PE-transposed to feature-major rawT[896, BS] in DRAM, plus
    per-column sum-of-squares (ScalarE accum on transposed tiles) and
    per-group linear sums for the BatchNorm statistics.
  Host: combine partial sums -> per-column affine (A, C) for all 990 flat
    features (BN is a global affine; gamma=1 so it commutes with max/min),
    fold into W1' = W1*A, b1' = b1 + W1@C.
  Phase 2 (device): h = relu(W1' @ rawT + b1') via chunked PE matmuls with
    features on the contraction axis (no transposes), y = u @ h + c0.
"""

import numpy as np

import concourse.bass as bass
import concourse.bacc as bacc
import concourse.mybir as mybir
from concourse.tile import TileContext
from concourse.bass_utils import run_bass_kernel_spmd

F32 = mybir.dt.float32
ALU = mybir.AluOpType
AX = mybir.AxisListType
AF = mybir.ActivationFunctionType

B = 131072
NCORES = 8
BS = B // NCORES            # 16384 samples per core
S = 8                       # sample-blocks per tile (each 128 samples)
PT = 128
TSAMP = PT * S              # 1024 samples per tile
NT = BS // TSAMP            # 16 tiles
NF, NW, ND = 11, 3, 10
XC = NF * NW * ND           # 330
RC = 896                    # padded raw feature columns (825 used; window-sum
                            # columns are folded into W1 on the host)
NCC = RC // PT              # 7 feature chunks
EPS = 1e-5

# base feature groups in F (165 features x 3 windows): feat ranges
GF = [(0, 55), (55, 110), (110, 121), (121, 132), (132, 143), (143, 154), (154, 165)]
GCNT = np.array([55, 55, 11, 11, 11, 11, 11], dtype=np.int64)
# (alpha, beta): true_feature = alpha*raw + beta
GAB = [(1.0, 0.0), (1.0 / 9.0, 0.0), (1.0 / 3.0, 0.0), (0.3, 0.0),
       (1.0 / 55.0, 0.0), (0.1, 0.0), (1.0, -1.0)]
# feature slices in F: corr, cov, std, z, dec, mean, ret
SL_CORR, SL_COV, SL_STD, SL_Z, SL_DEC, SL_MEAN, SL_RET = GF
# pair-block offsets for shifted products d=1..10 within 55-pair packing
OFFD = np.cumsum([0] + [11 - d for d in range(1, 11)]).tolist()  # OFFD[d-1] = start


def _feat_perm():
    """map my 165-feature index -> reference feature index (triu pair order)."""
    iu, ju = np.triu_indices(NF, k=1)
    ref_of_pair = {(i, j): k for k, (i, j) in enumerate(zip(iu, ju))}
    fmap = np.arange(165)
    for d in range(1, 11):
        for f in range(NF - d):
            mine = OFFD[d - 1] + f
            ref = ref_of_pair[(f, f + d)]
            fmap[mine] = ref           # corr block
            fmap[55 + mine] = 55 + ref  # cov block
    return fmap


FMAP = _feat_perm()
# stored raw cols: [0:495 base (3*fm+w)] [495:660 wmax] [660:825 wmin]
# (window-sum cols are not stored; their W1 weight folds onto the base cols)
COLMAP_BASE = np.zeros(495, dtype=np.int64)
for _fm in range(165):
    for _w in range(3):
        COLMAP_BASE[3 * _fm + _w] = 3 * FMAP[_fm] + _w

# my raw col -> BN group id (base 0..6; wsum 7+g via base sums; wmax 14+g,
# wmin 21+g), -1 for pad
gof = np.concatenate([np.full(GCNT[g], g) for g in range(7)])  # feat->group
COLGRP = np.full(RC, -1, dtype=np.int64)
COLGRP[0:495] = np.repeat(gof, 3)
COLGRP[495:660] = 14 + gof
COLGRP[660:825] = 21 + gof


def build_phase1():
    nc = bacc.Bacc()
    x_in = nc.dram_tensor("x_in", [BS, XC], F32, kind="ExternalInput")
    wfull_in = nc.dram_tensor("wfull_in", [PT, S * XC], F32, kind="ExternalInput")
    id_in = nc.dram_tensor("id_in", [PT, PT], F32, kind="ExternalInput")
    rawt_out = nc.dram_tensor("rawt_out", [RC, BS], F32, kind="ExternalOutput")
    psq_out = nc.dram_tensor("psq_out", [PT, NT, NCC], F32, kind="ExternalOutput")
    psl_out = nc.dram_tensor("psl_out", [PT, NT, NCC], F32, kind="ExternalOutput")
    psw_out = nc.dram_tensor("psw_out", [PT, NT, 7], F32, kind="ExternalOutput")

    # sample id = t*1024 + s*128 + p  (contiguous 128-sample runs per (t,s))
    x_t = x_in.rearrange("(t s p) c -> t p s c", t=NT, s=S, p=PT)
    # iteration order (cp, cc, j) matches the SBUF-side [p, cc, j] layout
    rt = rawt_out.rearrange("(cc cp) (t s j) -> t s cp cc j",
                            cc=NCC, cp=PT, t=NT, s=S)

    with TileContext(nc) as tc:
        with tc.tile_pool(name="cst", bufs=1) as cp, \
             tc.tile_pool(name="io", bufs=2) as iop, \
             tc.tile_pool(name="wk", bufs=1) as wp, \
             tc.tile_pool(name="big", bufs=2) as bp, \
             tc.tile_pool(name="ft", bufs=2) as fp_, \
             tc.tile_pool(name="tp", bufs=2, space="PSUM") as pp:

            wful = cp.tile([PT, S * XC], F32)
            nc.sync.dma_start(wful[:], wfull_in[:, :])
            IDT = cp.tile([PT, PT], F32)
            nc.sync.dma_start(IDT[:], id_in[:, :])
            SCR = cp.tile([PT, S * 55 * NW], F32)
            PSQ = cp.tile([PT, NT, NCC], F32)
            PSL = cp.tile([PT, NT, NCC], F32)
            PSW = cp.tile([PT, NT, 7], F32)
            Xt = []
            X0 = iop.tile([PT, S * XC], F32, tag="X")
            nc.sync.dma_start(X0.rearrange("p (s c) -> p s c", s=S), x_t[0])
            Xt.append(X0)
            for t in range(NT):
                if t + 1 < NT:
                    Xn = iop.tile([PT, S * XC], F32, tag="X")
                    nc.sync.dma_start(Xn.rearrange("p (s c) -> p s c", s=S),
                                      x_t[t + 1])
                    Xt.append(Xn)
                X = Xt[t]
                Xv = X.rearrange("p (s f w d) -> p s f w d", s=S, f=NF, w=NW, d=ND)
                X4 = X.rearrange("p (s f wd) -> p s f wd", s=S, f=NF, wd=NW * ND)

                RAW = fp_.tile([PT, S * RC], F32, tag="RAW")
                RV = RAW.rearrange("p (s c) -> p s c", s=S)
                F3 = RV[:, :, 0:495].rearrange("p s (f w) -> p s f w", w=NW)
                nc.vector.memset(RV[:, :, 825:RC], 0.0)

                mean3 = F3[:, :, SL_MEAN[0]:SL_MEAN[1], :]
                nc.vector.tensor_reduce(mean3, Xv, axis=AX.X, op=ALU.add)

                # squares -> s2 (squares on ScalarE, reduce on DVE)
                SQ = bp.tile([PT, S * XC], F32, tag="BIG")
                nc.scalar.activation(SQ[:], X[:], AF.Square)
                s2t = wp.tile([PT, S, NF, NW], F32, tag="s2t")
                nc.vector.tensor_reduce(
                    s2t[:], SQ.rearrange("p (s f w d) -> p s f w d",
                                         s=S, f=NF, w=NW, d=ND),
                    axis=AX.X, op=ALU.add)

                # shifted pair products d=1..10 -> SS [p,S,55,3]
                SS = wp.tile([PT, S, 55, NW], F32, tag="SS")
                for d in range(1, 11):
                    o = OFFD[d - 1]
                    PD = bp.tile([PT, S, NF - d, NW * ND], F32, tag="BIG")
                    nc.vector.tensor_tensor(PD[:], X4[:, :, 0:NF - d, :],
                                            X4[:, :, d:NF, :], ALU.mult)
                    nc.vector.tensor_reduce(
                        SS[:, :, o:o + NF - d, :],
                        PD.rearrange("p s f (w d) -> p s f w d", w=NW),
                        axis=AX.X, op=ALU.add)

                # scaled mean (0.1*mean') -> pre-scaled pair products
                MSC = wp.tile([PT, S, NF, NW], F32, tag="MSC")
                nc.vector.tensor_scalar_mul(MSC[:], mean3, 0.1)
                MM = wp.tile([PT, S, 55, NW], F32, tag="MM")
                for d in range(1, 11):
                    o = OFFD[d - 1]
                    nc.vector.tensor_tensor(MM[:, :, o:o + NF - d, :],
                                            MSC[:, :, 0:NF - d, :],
                                            F3[:, :, SL_MEAN[0] + d:SL_MEAN[1], :],
                                            ALU.mult)
                # var' = s2 - 0.1*mean'^2 ; cov' = SS - 0.1*MM
                VT = wp.tile([PT, S, NF, NW], F32, tag="VT")
                nc.vector.tensor_tensor(VT[:], MSC[:], mean3, ALU.mult)
                VARP = wp.tile([PT, S, NF, NW], F32, tag="VARP")
                nc.vector.tensor_tensor(VARP[:], s2t[:], VT[:], ALU.subtract)
                covF = F3[:, :, SL_COV[0]:SL_COV[1], :]
                nc.vector.tensor_tensor(covF, SS[:], MM[:], ALU.subtract)

                # dec = sum(x * d)
                DW = bp.tile([PT, S * XC], F32, tag="BIG")
                nc.vector.tensor_tensor(DW[:], X[:], wful[:], ALU.mult)
                nc.vector.tensor_reduce(
                    F3[:, :, SL_DEC[0]:SL_DEC[1], :],
                    DW.rearrange("p (s f w d) -> p s f w d", s=S, f=NF, w=NW, d=ND),
                    axis=AX.X, op=ALU.add)

                # std_raw = sqrt(var'), rstd = 1/std_raw
                stdF = F3[:, :, SL_STD[0]:SL_STD[1], :]
                nc.scalar.sqrt(stdF, VARP[:])
                RSTDf = wp.tile([PT, S, NF * NW], F32, tag="RSTD")
                RSC = wp.tile([PT, S, NF * NW], F32, tag="RSC")
                nc.vector.reciprocal_approx_accurate(
                    RSTDf[:], RV[:, :, 3 * SL_STD[0]:3 * SL_STD[1]], RSC[:])
                RSTD = RSTDf.rearrange("p s (f w) -> p s f w", w=NW)

                # corr = cov' * rstd_i * rstd_j
                RR = wp.tile([PT, S, 55, NW], F32, tag="RR")
                for d in range(1, 11):
                    o = OFFD[d - 1]
                    nc.vector.tensor_tensor(RR[:, :, o:o + NF - d, :],
                                            RSTD[:, :, 0:NF - d, :],
                                            RSTD[:, :, d:NF, :], ALU.mult)
                nc.vector.tensor_tensor(F3[:, :, SL_CORR[0]:SL_CORR[1], :],
                                        covF, RR[:], ALU.mult)

                # z = mean' * rstd
                nc.vector.tensor_tensor(F3[:, :, SL_Z[0]:SL_Z[1], :],
                                        mean3, RSTD[:], ALU.mult)

                # ret = x9 / x0
                R0f = wp.tile([PT, S, NF * NW], F32, tag="R0")
                nc.vector.reciprocal_approx_accurate(
                    R0f[:],
                    Xv[:, :, :, :, 0].rearrange("p s f w -> p s (f w)"), RSC[:])
                R0 = R0f.rearrange("p s (f w) -> p s f w", w=NW)
                nc.vector.tensor_tensor(F3[:, :, SL_RET[0]:SL_RET[1], :],
                                        R0[:], Xv[:, :, :, :, 9], ALU.mult)

                # window stats over the 3 windows: sum -> scratch (folded into
                # W1 on host, only its BN stats needed), max/min -> raw cols
                T01 = wp.tile([PT, S, 165], F32, tag="T01")
                WS = wp.tile([PT, S, 165], F32, tag="WS")
                for si, op in enumerate([ALU.add, ALU.max, ALU.min]):
                    dst = WS[:] if si == 0 else RV[:, :, 330 + 165 * si:495 + 165 * si]
                    nc.vector.tensor_tensor(T01[:], F3[:, :, :, 0], F3[:, :, :, 1], op)
                    nc.vector.tensor_tensor(dst, T01[:], F3[:, :, :, 2], op)
                # E[wsum^2] partial sums per group
                for g, (a, b) in enumerate(GF):
                    scr = SCR[:, 0:S * (b - a)].rearrange("p (s f) -> p s f", s=S)
                    nc.scalar.activation(scr, WS[:, :, a:b], AF.Square,
                                         accum_out=PSW[:, t, g:g + 1])

                # transpose RAW -> feature-major IN PLACE (the evacuated
                # transpose of slot s overwrites slot s's raw cols — same
                # 896 floats), then per-column sums + DMA out
                for s in range(S):
                    TP = pp.tile([PT, NCC * PT], F32, tag="TP")
                    for cc in range(NCC):
                        nc.tensor.transpose(TP[:, cc * PT:(cc + 1) * PT],
                                            RV[:, s, cc * PT:(cc + 1) * PT], IDT[:])
                    nc.scalar.copy(RV[:, s, :], TP[:])
                    nc.sync.dma_start(
                        rt[t, s],
                        RV[:, s, :].rearrange("p (cc j) -> p cc j", cc=NCC))
                for cc in range(NCC):
                    scr = SCR[:, 0:S * PT].rearrange("p (s j) -> p s j", s=S)
                    nc.scalar.activation(scr, RV[:, :, cc * PT:(cc + 1) * PT],
                                         AF.Square,
                                         accum_out=PSQ[:, t, cc:cc + 1])
                    nc.scalar.activation(scr, RV[:, :, cc * PT:(cc + 1) * PT],
                                         AF.Copy,
                                         accum_out=PSL[:, t, cc:cc + 1])
            nc.sync.dma_start(psq_out[:, :, :], PSQ[:])
            nc.sync.dma_start(psl_out[:, :, :], PSL[:])
            nc.sync.dma_start(psw_out[:, :, :], PSW[:])
    return nc


def build_phase2():
    nc = bacc.Bacc()
    rawt_in = nc.dram_tensor("rawt_in", [RC, BS], F32, kind="ExternalInput")
    w1t_in = nc.dram_tensor("w1t_in", [RC, 32], F32, kind="ExternalInput")
    b1_in = nc.dram_tensor("b1_in", [32, 1], F32, kind="ExternalInput")
    u_in = nc.dram_tensor("u_in", [32, 1], F32, kind="ExternalInput")
    c0_in = nc.dram_tensor("c0_in", [1, 1], F32, kind="ExternalInput")
    y_out = nc.dram_tensor("y_out", [1, BS], F32, kind="ExternalOutput")

    NB = BS // 512  # 32 blocks of 512 samples
    rtb = rawt_in.rearrange("(cc cp) (n j) -> n cp cc j", cc=NCC, cp=PT, n=NB)

    with TileContext(nc) as tc:
        with tc.tile_pool(name="cst", bufs=1) as cp, \
             tc.tile_pool(name="sb", bufs=3) as sp, \
             tc.tile_pool(name="ps", bufs=4, space="PSUM") as pp:
            W1S = cp.tile([PT, NCC * 32], F32)
            W1Sv = W1S.rearrange("p (c m) -> p c m", c=NCC)
            nc.sync.dma_start(W1Sv,
                              w1t_in.rearrange("(c p) m -> p c m", c=NCC, p=PT))
            B1T = cp.tile([32, 1], F32)
            nc.sync.dma_start(B1T[:], b1_in[:, :])
            UT = cp.tile([32, 1], F32)
            nc.sync.dma_start(UT[:], u_in[:, :])
            C0T = cp.tile([1, 1], F32)
            nc.sync.dma_start(C0T[:], c0_in[:, :])

            for n in range(NB):
                RT = sp.tile([PT, NCC, 512], F32, tag="RT")
                nc.gpsimd.dma_start(RT[:], rtb[n])
                HP = pp.tile([32, 512], F32, tag="HP")
                for cc in range(NCC):
                    nc.tensor.matmul(HP[:], W1Sv[:, cc, :], RT[:, cc, :],
                                     start=(cc == 0), stop=(cc == NCC - 1))
                HS = sp.tile([32, 512], F32, tag="HS")
                nc.scalar.activation(HS[:], HP[:], AF.Relu, bias=B1T[:, 0:1], scale=1.0)
                OP = pp.tile([1, 512], F32, tag="OP")
                nc.tensor.matmul(OP[:], UT[:], HS[:], start=True, stop=True)
                OS = sp.tile([1, 512], F32, tag="OS")
                nc.vector.tensor_scalar(OS[:], OP[:], C0T[0:1, 0:1], None, ALU.add)
                nc.gpsimd.dma_start(y_out[0:1, n * 512:(n + 1) * 512], OS[:])
    return nc


_CACHE = {}
LAST_EXEC_NS = {}


def _run(nc, in_maps, **kw):
    import os
    tr = os.environ.get("KTRACE", "") == "1"
    if tr:
        kw.setdefault("trace", True)
    return run_bass_kernel_spmd(nc, in_maps, **kw)


def _get_kernels():
    if "p1" not in _CACHE:
        _CACHE["p1"] = build_phase1()
        _CACHE["p1"].finalize()
        _CACHE["p2"] = build_phase2()
        _CACHE["p2"].finalize()
    return _CACHE["p1"], _CACHE["p2"]


def kernel(x, gamma, beta, W1, b1, W2, b2, w_scale, b_scale):
    x = np.asarray(x, dtype=np.float32)
    W1 = np.asarray(W1, np.float32); b1 = np.asarray(b1, np.float32)
    W2 = np.asarray(W2, np.float32); b2 = np.asarray(b2, np.float32)
    gamma_f = float(np.asarray(gamma).reshape(-1)[0])
    beta_f = float(np.asarray(beta).reshape(-1)[0])
    wsc = float(np.asarray(w_scale).reshape(-1)[0])
    bsc = float(np.asarray(b_scale).reshape(-1)[0])

    nc1, nc2 = _get_kernels()
    xs = np.ascontiguousarray(x.reshape(B, XC))
    wbase = np.tile(np.arange(1, 11, dtype=np.float32), NF * NW)  # [330]
    wfull = np.tile(wbase, (PT, S))
    ident = np.eye(PT, dtype=np.float32)

    in1 = [{"x_in": xs[c * BS:(c + 1) * BS], "wfull_in": wfull, "id_in": ident}
           for c in range(NCORES)]
    r1 = _run(nc1, in1, core_ids=list(range(NCORES)))
    LAST_EXEC_NS["p1"] = r1.exec_time_ns
    rawts = [r["rawt_out"] for r in r1.results]
    # per-column sums / squared sums (col id = cc*128 + p) -> group sums
    csq = np.zeros(RC, np.float64)
    clin = np.zeros(RC, np.float64)
    psw = np.zeros(7, np.float64)
    for r in r1.results:
        csq += r["psq_out"].astype(np.float64).sum(axis=1).T.reshape(-1)
        clin += r["psl_out"].astype(np.float64).sum(axis=1).T.reshape(-1)
        psw += r["psw_out"].astype(np.float64).sum(axis=(0, 1))
    P = np.zeros(64, np.float64)
    for g in range(7):
        P[g] = csq[COLGRP == g].sum()            # base sum(raw^2)
        P[7 + g] = clin[COLGRP == g].sum()       # base sum(raw) = sum(wsum)
        P[14 + g] = clin[COLGRP == 14 + g].sum()  # wmax linear
        P[21 + g] = clin[COLGRP == 21 + g].sum()  # wmin linear
        P[28 + g] = psw[g]                        # wsum squared
        P[35 + g] = csq[COLGRP == 14 + g].sum()   # wmax squared
        P[42 + g] = csq[COLGRP == 21 + g].sum()   # wmin squared

    # base group BN affines
    A_base = np.zeros(7); C_base = np.zeros(7)
    for g in range(7):
        alpha, bet = GAB[g]
        N = float(B * GCNT[g] * 3)
        S1 = P[7 + g]          # sum of raw (= sum of wsum over group)
        S2 = P[g]              # sum of raw^2
        mT = (alpha * S1 + bet * N) / N
        e2 = (alpha * alpha * S2 + 2 * alpha * bet * S1 + bet * bet * N) / N
        v = e2 - mT * mT
        a = gamma_f / np.sqrt(v + EPS)
        c = beta_f - a * mT
        A_base[g] = a * alpha
        C_base[g] = a * bet + c

    # second-level BN affines (wsum/3, wmax, wmin; p1 = groups 0..5, p2 = {6})
    A2 = np.zeros((3, 7)); C2 = np.zeros((3, 7))
    for si in range(3):
        k = A_base * (1.0 / 3.0 if si == 0 else 1.0)
        off = C_base
        S1g = P[7 + 7 * si:14 + 7 * si].copy()
        S2g = P[28 + 7 * si:35 + 7 * si].copy()
        for grp_set, idxs in (("p1", range(6)), ("p2", [6])):
            Ntot = float(B * sum(GCNT[i] for i in idxs))
            m = sum(k[i] * S1g[i] + B * GCNT[i] * off[i] for i in idxs) / Ntot
            e2 = sum(k[i] ** 2 * S2g[i] + 2 * k[i] * off[i] * S1g[i]
                     + B * GCNT[i] * off[i] ** 2 for i in idxs) / Ntot
            v = e2 - m * m
            a2 = gamma_f / np.sqrt(v + EPS)
            c2 = beta_f - a2 * m
            for i in idxs:
                A2[si, i] = a2 * k[i]
                C2[si, i] = a2 * off[i] + c2

    # reference-order affine over the 990 flat columns (group-constant, so the
    # same values apply in my column order)
    A_ref = np.zeros(990); C_ref = np.zeros(990)
    A_ref[0:495] = np.repeat(A_base[gof], 3)
    C_ref[0:495] = np.repeat(C_base[gof], 3)
    for si in range(3):
        A_ref[495 + 165 * si:660 + 165 * si] = A2[si, gof]
        C_ref[495 + 165 * si:660 + 165 * si] = C2[si, gof]

    # W1 fold: base cols carry their own affine plus the wsum-column weight
    W1A = np.zeros((32, RC), np.float32)
    W1base = W1[:, COLMAP_BASE]                 # [30, 495] my order
    W1ws = W1[:, 495 + FMAP]                    # [30, 165] my feature order
    W1A[:30, 0:495] = (W1base * A_ref[None, 0:495]
                       + np.repeat(W1ws * A2[0, gof][None, :], 3, axis=1)
                       ).astype(np.float32)
    W1A[:30, 495:660] = (W1[:, 660 + FMAP] * A2[1, gof][None, :]).astype(np.float32)
    W1A[:30, 660:825] = (W1[:, 825 + FMAP] * A2[2, gof][None, :]).astype(np.float32)
    b1p = np.zeros((32, 1), np.float32)
    b1p[:30, 0] = b1 + (W1 @ C_ref).astype(np.float32)
    u = np.zeros((32, 1), np.float32)
    u[:30, 0] = wsc * W2[0]
    c0 = np.float32(wsc * float(b2[0]) + bsc)

    in2 = [{"rawt_in": rawts[c], "w1t_in": np.ascontiguousarray(W1A.T),
            "b1_in": b1p, "u_in": u, "c0_in": np.array([[c0]], np.float32)}
           for c in range(NCORES)]
    r2 = _run(nc2, in2, core_ids=list(range(NCORES)))
    LAST_EXEC_NS["p2"] = r2.exec_time_ns
    # sample id within core = t*1024 + s*128 + p == linear index (identity)
    y = np.concatenate([r["y_out"][0] for r in r2.results])
    return y.astype(np.float32)
